# revision 13
# baseline (speedup 1.0000x reference)
"""Trainium2 Bass kernel for causal multi-head attention (8-core SPMD).

Problem: B=2, S=2048, H=2048, 16 heads (hd=128), RoPE, causal mask,
layer-index scaling (/4), additive pad mask (zeros by construction).

Sharding: core c handles batch b=c//4 and head-group g=c%4 (4 heads).
wq/wk/wv column-parallel, wo row-parallel; host sums the 4 partial
outputs per batch.

Per-core dataflow (all feature-on-partition, "transposed" layouts):
  qT/kT [d=128, S] = w.T-tile @ xT      (PSUM accum over 16 H-chunks)
  RoPE on qT/kT via head-dim permutation chosen so the rotation pair
    sits 16 partitions apart inside each 32-partition quadrant
    (stream_shuffle does the swap in one DVE op)
  scoresT [k,q] tiles = kT-tile.T @ qT-block ; exp on ACT -> PT (bf16)
  diagonal tiles masked multiplicatively post-exp
  row sums via all-ones stationary matmul (broadcast over partitions)
  OT [d, q] += v-tile.T @ PT ; normalized by reciprocal(sums)
  out_partial = OT.T @ woT  (PSUM accum over the 4 local head chunks)

Matmuls run in bf16 (fp32 PSUM accumulation); softmax math in fp32.
"""
import math
import os
import sys

import numpy as np

for _p in ("/opt/trn_rl_repo", "/root/.axon_site/_ro/trn_rl_repo"):
    if os.path.isdir(_p) and _p not in sys.path:
        sys.path.append(_p)

import ml_dtypes

S = 2048
H = 2048
NHEADS = 16
HD = 128
NH_LOC = 4          # heads per core
D_LOC = NH_LOC * HD  # 512
LAYER_INDEX = 3
SCALE = 1.0 / (math.sqrt(HD) * (LAYER_INDEX + 1))
N_CORES = 8
SB = 512            # S-block (matmul moving free dim)
HC = H // 128       # contraction chunks

# head-dim permutation: RoPE pair (x1_j, x2_j) -> rows (qd*32 + j%16,
# qd*32 + 16 + j%16) with qd = j//16, so the swap is within-quadrant.
_P_NEW2OLD = np.zeros(HD, dtype=np.int64)
_J_OF_P = np.zeros(HD, dtype=np.int64)
_SIGN_OF_P = np.zeros(HD, dtype=np.float32)
for _p in range(HD):
    _qd, _r = _p // 32, _p % 32
    _j = _qd * 16 + (_r % 16)
    _P_NEW2OLD[_p] = 2 * _j + (1 if _r >= 16 else 0)
    _J_OF_P[_p] = _j
    _SIGN_OF_P[_p] = 1.0 if _r >= 16 else -1.0
_SHUF_MASK = [(i + 16) % 32 for i in range(32)]

_BF16 = ml_dtypes.bfloat16
_NC_CACHE = {}


def _build_nc():
    import concourse.bacc as bacc
    import concourse.mybir as mybir
    import concourse.tile as tile

    f32 = mybir.dt.float32
    bf16 = mybir.dt.bfloat16
    Exp = mybir.ActivationFunctionType.Exp

    nc = bacc.Bacc("TRN2", target_bir_lowering=False, debug=False)

    xt_d = nc.dram_tensor("xt", [H, S], bf16, kind="ExternalInput")
    wqt_d = nc.dram_tensor("wqt", [H, D_LOC], bf16, kind="ExternalInput")
    wkt_d = nc.dram_tensor("wkt", [H, D_LOC], bf16, kind="ExternalInput")
    wvt_d = nc.dram_tensor("wvt", [H, D_LOC], bf16, kind="ExternalInput")
    wot_d = nc.dram_tensor("wot", [D_LOC, H], bf16, kind="ExternalInput")
    cos_d = nc.dram_tensor("cos_bc", [128, S], f32, kind="ExternalInput")
    sin_d = nc.dram_tensor("sin_pm", [128, S], f32, kind="ExternalInput")
    masks_d = nc.dram_tensor("masks", [4, 128, SB], bf16, kind="ExternalInput")
    ones_d = nc.dram_tensor("ones", [128, 128], bf16, kind="ExternalInput")
    out_d = nc.dram_tensor("out_partial", [S, H], f32, kind="ExternalOutput")

    n_sb = S // SB       # 4
    n_st = S // 128      # 16

    with tile.TileContext(nc) as tc:
        with (
            tc.tile_pool(name="const", bufs=1) as const_pool,
            tc.tile_pool(name="qkv", bufs=1) as qkv_pool,
        ):
            cos_t = const_pool.tile([128, S], f32, tag="cos")
            sin_t = const_pool.tile([128, S], f32, tag="sin")
            mask_t = const_pool.tile([128, 4, SB], bf16, tag="mask")
            ones_t = const_pool.tile([128, 128], bf16, tag="ones")

            qT = qkv_pool.tile([128, NH_LOC, S], bf16, tag="qT")
            kT = qkv_pool.tile([128, NH_LOC, S], bf16, tag="kT")
            v_t = qkv_pool.tile([128, n_st, D_LOC], bf16, tag="v")

            # ---------------- Phase A: projections + RoPE ----------------
            with (
                tc.tile_pool(name="w", bufs=1) as w_pool,
                tc.tile_pool(name="xtp", bufs=2) as xt_pool,
                tc.tile_pool(name="rope", bufs=2) as rope_pool,
                tc.tile_pool(name="psA", bufs=2, space="PSUM") as psA,
            ):
                wq_t = w_pool.tile([128, HC, D_LOC], bf16, tag="wq")
                wk_t = w_pool.tile([128, HC, D_LOC], bf16, tag="wk")
                wv_t = w_pool.tile([128, HC, D_LOC], bf16, tag="wv")

                xt_view = xt_d[:, :].rearrange(
                    "(hc p) (sb f) -> sb p hc f", p=128, f=SB)
                wq_view = wqt_d[:, :].rearrange("(hc p) d -> p hc d", p=128)
                wk_view = wkt_d[:, :].rearrange("(hc p) d -> p hc d", p=128)
                wv_view = wvt_d[:, :].rearrange("(hc p) d -> p hc d", p=128)

                xt_blk0 = xt_pool.tile([128, HC, SB], bf16, tag="xt")
                # chunked loads, interleaved so hc=0.. arrives first and the
                # first projection matmuls start ~immediately (subtile deps);
                # streams ordered by first use: xt+wq, cos/sin, wk, wv
                for c0 in range(0, HC, 2):
                    csl = slice(c0, c0 + 2)
                    nc.sync.dma_start(xt_blk0[:, csl, :], xt_view[0][:, csl, :])
                    nc.gpsimd.dma_start(wq_t[:, csl, :], wq_view[:, csl, :])
                nc.scalar.dma_start(cos_t[:], cos_d[:, :])
                nc.scalar.dma_start(sin_t[:], sin_d[:, :])
                for c0 in range(0, HC, 4):
                    csl = slice(c0, c0 + 4)
                    nc.gpsimd.dma_start(wk_t[:, csl, :], wk_view[:, csl, :])
                for c0 in range(0, HC, 4):
                    csl = slice(c0, c0 + 4)
                    nc.gpsimd.dma_start(wv_t[:, csl, :], wv_view[:, csl, :])
                nc.gpsimd.dma_start(
                    mask_t[:], masks_d[:, :, :].rearrange("j p f -> p j f"))
                nc.gpsimd.dma_start(ones_t[:], ones_d[:, :])

                for sb in range(n_sb):
                    ssl = slice(sb * SB, (sb + 1) * SB)
                    if sb == 0:
                        xt_blk = xt_blk0
                    else:
                        xt_blk = xt_pool.tile([128, HC, SB], bf16, tag="xt")
                        for c0 in range(0, HC, 4):
                            csl = slice(c0, c0 + 4)
                            nc.sync.dma_start(
                                xt_blk[:, csl, :], xt_view[sb][:, csl, :])

                    for w_tile, dst in ((wq_t, qT), (wk_t, kT)):
                        # hc-outer: PE consumes DMA chunks incrementally,
                        # 4 PSUM banks accumulate the 4 heads in parallel
                        ps_h = [psA.tile([128, SB], f32, tag="pqk", bufs=6,
                                         name=f"pqk{h}")
                                for h in range(NH_LOC)]
                        for hc in range(HC):
                            for h in range(NH_LOC):
                                hs = slice(h * 128, (h + 1) * 128)
                                nc.tensor.matmul(
                                    ps_h[h][:], w_tile[:, hc, hs],
                                    xt_blk[:, hc, :],
                                    start=(hc == 0), stop=(hc == HC - 1))
                        for h in range(NH_LOC):
                            # RoPE: dst = ps*cos + shuffle(ps)*sin_pm
                            ps = ps_h[h]
                            t_sw = rope_pool.tile([128, SB], f32, tag="sw")
                            nc.vector.stream_shuffle(t_sw[:], ps[:], _SHUF_MASK)
                            t_pr = rope_pool.tile([128, SB], f32, tag="pr")
                            nc.vector.tensor_mul(t_pr[:], t_sw[:], sin_t[:, ssl])
                            t_cs = rope_pool.tile([128, SB], f32, tag="cs")
                            nc.vector.tensor_mul(t_cs[:], ps[:], cos_t[:, ssl])
                            nc.vector.tensor_add(dst[:, h, ssl], t_cs[:], t_pr[:])

                    for i in range(n_sb):
                        st = sb * 4 + i
                        isl = slice(i * 128, (i + 1) * 128)
                        ps = psA.tile([128, D_LOC], f32, tag="pv", bufs=2)
                        for hc in range(HC):
                            nc.tensor.matmul(
                                ps[:], xt_blk[:, hc, isl], wv_t[:, hc, :],
                                start=(hc == 0), stop=(hc == HC - 1))
                        nc.scalar.copy(v_t[:, st, :], ps[:])

            # ------------- Phase B: attention, Phase C: out proj -------------
            with (
                tc.tile_pool(name="wo", bufs=1) as wo_pool,
                tc.tile_pool(name="ot", bufs=1) as ot_pool,
            ):
                wo_t = wo_pool.tile([128, NH_LOC, H], bf16, tag="wo")
                nc.sync.dma_start(
                    wo_t[:], wot_d[:, :].rearrange("(dc p) o -> p dc o", p=128))
                ot_t = ot_pool.tile([128, NH_LOC, S], bf16, tag="ot")

                with (
                    tc.tile_pool(name="pt", bufs=2) as pt_pool,
                    tc.tile_pool(name="scr", bufs=2) as scr_pool,
                    tc.tile_pool(name="rcp", bufs=2) as rcp_pool,
                    tc.tile_pool(name="stage", bufs=4) as stage_pool,
                    tc.tile_pool(name="psB", bufs=1, space="PSUM") as psB,
                ):
                    # phase-C work units (st, hb), emitted interleaved with
                    # phase B so the in-order PE has filler during exp waits
                    c_units = []

                    def emit_c_unit(use_scalar=False):
                        st, hb = c_units.pop(0)
                        stsl = slice(st * 128, (st + 1) * 128)
                        ps_c = psB.tile([128, SB], f32, tag="pc", bufs=2,
                                        name=f"pc_{st}_{hb}")
                        for dc in range(NH_LOC):
                            nc.tensor.matmul(
                                ps_c[:],
                                ot_t[:, dc, stsl],
                                wo_t[:, dc, hb * SB:(hb + 1) * SB],
                                start=(dc == 0), stop=(dc == NH_LOC - 1))
                        o_sb = stage_pool.tile([128, SB], f32, tag="st")
                        if use_scalar:
                            nc.scalar.copy(o_sb[:], ps_c[:])
                        else:
                            nc.vector.tensor_copy(o_sb[:], ps_c[:])
                        nc.sync.dma_start(
                            out_d[stsl, hb * SB:(hb + 1) * SB], o_sb[:])

                    tri = mask_t[:, 0, 0:128]  # keep f >= p triangle
                    for qb in range(n_sb):
                        qsl = slice(qb * SB, (qb + 1) * SB)
                        nkt = 4 * (qb + 1)
                        for h in range(NH_LOC):
                            hs = slice(h * 128, (h + 1) * 128)
                            blk = pt_pool.tile([128, 16, SB], bf16, tag="pt")
                            ps_o = psB.tile([128, SB], f32, tag="o", bufs=2)
                            for c0 in range(0, nkt, 4):
                                for kt in range(c0, c0 + 4):
                                    j = kt - 4 * qb
                                    off = 128 * j if j > 0 else 0
                                    W = SB - off
                                    ksl = slice(kt * 128, (kt + 1) * 128)
                                    ps_s = psB.tile(
                                        [128, SB], f32, tag="s", bufs=3)
                                    nc.tensor.matmul(
                                        ps_s[:, 0:W], kT[:, h, ksl],
                                        qT[:, h, qb * SB + off:(qb + 1) * SB],
                                        start=True, stop=True)
                                    nc.scalar.activation(
                                        blk[:, kt, off:SB], ps_s[:, 0:W], Exp)
                                    if j >= 0:
                                        nc.vector.tensor_mul(
                                            blk[:, kt, off:off + 128],
                                            blk[:, kt, off:off + 128], tri)
                                for kt in range(c0, c0 + 4):
                                    j = kt - 4 * qb
                                    off = 128 * j if j > 0 else 0
                                    nc.tensor.matmul(
                                        ps_o[:, off:SB], v_t[:, kt, hs],
                                        blk[:, kt, off:SB],
                                        start=(kt == 0), stop=(kt == nkt - 1))
                                # phase-C filler for the PE during exp waits;
                                # pace so the backlog lasts through qb=3
                                n_fill = 1 if qb == 3 else 2
                                for _ in range(n_fill):
                                    if c_units:
                                        emit_c_unit()

                            # sums: elementwise kt-tree on DVE (bf16), then
                            # one all-ones matmul reduces partitions+broadcasts
                            scr = scr_pool.tile([128, 12, SB], bf16, tag="scr")
                            nd = nkt - 4  # non-diagonal count
                            # fold diag j=1..3 into diag j=0 (valid suffixes)
                            d0 = nkt - 4 + 0
                            for j in range(1, 4):
                                o = 128 * j
                                nc.vector.tensor_add(
                                    blk[:, d0, o:SB], blk[:, d0, o:SB],
                                    blk[:, d0 + j, o:SB])
                            if nd == 0:
                                sums_src = blk[:, d0, :]
                            else:
                                # pairwise-halve the nd non-diag tiles
                                nc.vector.tensor_add(
                                    scr[:, 0:nd // 2, :],
                                    blk[:, 0:nd:2, :], blk[:, 1:nd:2, :])
                                m = nd // 2
                                base = 0
                                while m > 1:
                                    nb = base + m
                                    nc.vector.tensor_add(
                                        scr[:, nb:nb + m // 2, :],
                                        scr[:, base:base + m - 1:2, :],
                                        scr[:, base + 1:base + m:2, :])
                                    if m % 2:
                                        # carry odd leftover
                                        nc.vector.tensor_add(
                                            scr[:, nb, :], scr[:, nb, :],
                                            scr[:, base + m - 1, :])
                                    base, m = nb, m // 2
                                nc.vector.tensor_add(
                                    scr[:, base, :], scr[:, base, :],
                                    blk[:, d0, :])
                                sums_src = scr[:, base, :]
                            ps_sum = psB.tile([128, SB], f32, tag="sum",
                                              bufs=1)
                            nc.tensor.matmul(ps_sum[:], ones_t[:],
                                             sums_src, start=True, stop=True)
                            rcp = rcp_pool.tile([128, SB], f32, tag="rcp")
                            nc.vector.reciprocal_approx_fast(rcp[:], ps_sum[:])
                            nc.vector.tensor_mul(
                                ot_t[:, h, qsl], ps_o[:], rcp[:])
                        # this qb's output rows are now fully available
                        for st in range(qb * 4, qb * 4 + 4):
                            for hb in range(4):
                                c_units.append((st, hb))
                    drain_i = 0
                    while c_units:
                        emit_c_unit(use_scalar=(drain_i % 2 == 0))
                        drain_i += 1

    nc.compile()
    return nc


def _host_prep(x, freq_cos, freq_sin, wq, wk, wv, wo):
    """Build the 8 per-core input maps."""
    cos_bc = np.ascontiguousarray(freq_cos.T[_J_OF_P, :]).astype(np.float32)
    sin_pm = np.ascontiguousarray(
        freq_sin.T[_J_OF_P, :] * _SIGN_OF_P[:, None]).astype(np.float32)

    f = np.arange(SB)[None, :]
    p = np.arange(128)[:, None]
    masks = np.stack(
        [(f - 128 * j - p >= 0) for j in range(4)]).astype(_BF16)
    ones = np.ones((128, 128), dtype=_BF16)

    xt_b = [np.ascontiguousarray(x[b].T).astype(_BF16) for b in range(2)]

    in_maps = []
    for c in range(N_CORES):
        b, g = c // 4, c % 4
        rows = slice(g * D_LOC, (g + 1) * D_LOC)
        wq_g = wq[rows, :].reshape(NH_LOC, HD, H)[:, _P_NEW2OLD, :]
        wk_g = wk[rows, :].reshape(NH_LOC, HD, H)[:, _P_NEW2OLD, :]
        in_maps.append({
            "xt": xt_b[b],
            "wqt": np.ascontiguousarray(
                wq_g.reshape(D_LOC, H).T * SCALE).astype(_BF16),
            "wkt": np.ascontiguousarray(
                wk_g.reshape(D_LOC, H).T).astype(_BF16),
            "wvt": np.ascontiguousarray(wv[rows, :].T).astype(_BF16),
            "wot": np.ascontiguousarray(wo[:, rows].T).astype(_BF16),
            "cos_bc": cos_bc,
            "sin_pm": sin_pm,
            "masks": masks,
            "ones": ones,
        })
    return in_maps


def _kernel_np_fallback(x, freq_cos, freq_sin, attention_mask, wq, wk, wv, wo):
    """Numpy fallback (only used if attention_mask is nonzero)."""
    B = x.shape[0]
    hd = H // NHEADS
    q = (x @ wq.T).reshape(B, S, NHEADS, hd)
    k = (x @ wk.T).reshape(B, S, NHEADS, hd)
    v = (x @ wv.T).reshape(B, S, NHEADS, hd)

    def rope(t):
        x1, x2 = t[..., ::2], t[..., 1::2]
        c = freq_cos[None, :, None, :]
        s = freq_sin[None, :, None, :]
        o = np.empty_like(t)
        o[..., ::2] = x1 * c - x2 * s
        o[..., 1::2] = x1 * s + x2 * c
        return o

    q, k = rope(q), rope(k)
    q = q.transpose(0, 2, 1, 3)
    k = k.transpose(0, 2, 1, 3)
    v = v.transpose(0, 2, 1, 3)
    att = np.einsum("bhqd,bhkd->bhqk", q, k) / np.sqrt(hd) / (LAYER_INDEX + 1)
    att = att + attention_mask
    causal = np.triu(np.full((S, S), -1e30, dtype=att.dtype), k=1)
    att = att + causal[None, None]
    att = att - att.max(axis=-1, keepdims=True)
    att = np.exp(att)
    att = att / att.sum(axis=-1, keepdims=True)
    out = np.einsum("bhqk,bhkd->bhqd", att, v)
    out = out.transpose(0, 2, 1, 3).reshape(B, S, H)
    return (out @ wo.T).astype(np.float32)


def kernel(x, freq_cos, freq_sin, attention_mask, wq, wk, wv, wo, **extra):
    x = np.asarray(x, dtype=np.float32)
    freq_cos = np.asarray(freq_cos, dtype=np.float32)
    freq_sin = np.asarray(freq_sin, dtype=np.float32)
    attention_mask = np.asarray(attention_mask, dtype=np.float32)
    wq = np.asarray(wq, dtype=np.float32)
    wk = np.asarray(wk, dtype=np.float32)
    wv = np.asarray(wv, dtype=np.float32)
    wo = np.asarray(wo, dtype=np.float32)

    if attention_mask.any():
        # the device kernel folds the (all-zero) pad mask away
        return _kernel_np_fallback(
            x, freq_cos, freq_sin, attention_mask, wq, wk, wv, wo)

    from concourse.bass_utils import run_bass_kernel_spmd

    if "nc" not in _NC_CACHE:
        _NC_CACHE["nc"] = _build_nc()
    nc = _NC_CACHE["nc"]

    in_maps = _host_prep(x, freq_cos, freq_sin, wq, wk, wv, wo)
    res = run_bass_kernel_spmd(nc, in_maps, list(range(N_CORES)))

    out = np.zeros((2, S, H), dtype=np.float32)
    for c in range(N_CORES):
        out[c // 4] += res.results[c]["out_partial"]
    return out


# revision 14
# speedup vs baseline: 1.0456x; 1.0456x over previous
"""Trainium2 Bass kernel for causal multi-head attention (8-core SPMD).

Problem: B=2, S=2048, H=2048, 16 heads (hd=128), RoPE, causal mask,
layer-index scaling (/4), additive pad mask (zeros by construction).

Sharding: core c handles batch b=c//4 and head-group g=c%4 (4 heads).
wq/wk/wv column-parallel, wo row-parallel; host sums the 4 partial
outputs per batch.

Per-core dataflow (all feature-on-partition, "transposed" layouts):
  qT/kT [d=128, S] = w.T-tile @ xT      (PSUM accum over 16 H-chunks)
  RoPE on qT/kT via head-dim permutation chosen so the rotation pair
    sits 16 partitions apart inside each 32-partition quadrant
    (stream_shuffle does the swap in one DVE op)
  scoresT [k,q] tiles = kT-tile.T @ qT-block ; exp on ACT -> PT (bf16)
  diagonal tiles masked multiplicatively post-exp
  row sums via all-ones stationary matmul (broadcast over partitions)
  OT [d, q] += v-tile.T @ PT ; normalized by reciprocal(sums)
  out_partial = OT.T @ woT  (PSUM accum over the 4 local head chunks)

Matmuls run in bf16 (fp32 PSUM accumulation); softmax math in fp32.
"""
import math
import os
import sys

import numpy as np

for _p in ("/opt/trn_rl_repo", "/root/.axon_site/_ro/trn_rl_repo"):
    if os.path.isdir(_p) and _p not in sys.path:
        sys.path.append(_p)

import ml_dtypes

S = 2048
H = 2048
NHEADS = 16
HD = 128
NH_LOC = 4          # heads per core
D_LOC = NH_LOC * HD  # 512
LAYER_INDEX = 3
SCALE = 1.0 / (math.sqrt(HD) * (LAYER_INDEX + 1))
N_CORES = 8
SB = 512            # S-block (matmul moving free dim)
HC = H // 128       # contraction chunks

# head-dim permutation: RoPE pair (x1_j, x2_j) -> rows (qd*32 + j%16,
# qd*32 + 16 + j%16) with qd = j//16, so the swap is within-quadrant.
_P_NEW2OLD = np.zeros(HD, dtype=np.int64)
_J_OF_P = np.zeros(HD, dtype=np.int64)
_SIGN_OF_P = np.zeros(HD, dtype=np.float32)
for _p in range(HD):
    _qd, _r = _p // 32, _p % 32
    _j = _qd * 16 + (_r % 16)
    _P_NEW2OLD[_p] = 2 * _j + (1 if _r >= 16 else 0)
    _J_OF_P[_p] = _j
    _SIGN_OF_P[_p] = 1.0 if _r >= 16 else -1.0
_SHUF_MASK = [(i + 16) % 32 for i in range(32)]

_BF16 = ml_dtypes.bfloat16
_NC_CACHE = {}


def _build_nc():
    import concourse.bacc as bacc
    import concourse.mybir as mybir
    import concourse.tile as tile

    f32 = mybir.dt.float32
    bf16 = mybir.dt.bfloat16
    Exp = mybir.ActivationFunctionType.Exp

    nc = bacc.Bacc("TRN2", target_bir_lowering=False, debug=False)

    xt_d = nc.dram_tensor("xt", [H, S], bf16, kind="ExternalInput")
    wqt_d = nc.dram_tensor("wqt", [H, D_LOC], bf16, kind="ExternalInput")
    wkt_d = nc.dram_tensor("wkt", [H, D_LOC], bf16, kind="ExternalInput")
    wvt_d = nc.dram_tensor("wvt", [H, D_LOC], bf16, kind="ExternalInput")
    wot_d = nc.dram_tensor("wot", [D_LOC, H], bf16, kind="ExternalInput")
    cos_d = nc.dram_tensor("cos_bc", [128, S], f32, kind="ExternalInput")
    sin_d = nc.dram_tensor("sin_pm", [128, S], f32, kind="ExternalInput")
    masks_d = nc.dram_tensor("masks", [4, 128, SB], bf16, kind="ExternalInput")
    ones_d = nc.dram_tensor("ones", [128, 128], bf16, kind="ExternalInput")
    out_d = nc.dram_tensor("out_partial", [S, H], f32, kind="ExternalOutput")

    n_sb = S // SB       # 4
    n_st = S // 128      # 16

    with tile.TileContext(nc) as tc:
        with (
            tc.tile_pool(name="const", bufs=1) as const_pool,
            tc.tile_pool(name="qkv", bufs=1) as qkv_pool,
        ):
            cos_t = const_pool.tile([128, S], f32, tag="cos")
            sin_t = const_pool.tile([128, S], f32, tag="sin")
            mask_t = const_pool.tile([128, 4, SB], bf16, tag="mask")
            ones_t = const_pool.tile([128, 128], bf16, tag="ones")

            qT = qkv_pool.tile([128, NH_LOC, S], bf16, tag="qT")
            kT = qkv_pool.tile([128, NH_LOC, S], bf16, tag="kT")
            v_t = qkv_pool.tile([128, n_st, D_LOC], bf16, tag="v")

            # ---------------- Phase A: projections + RoPE ----------------
            with (
                tc.tile_pool(name="w", bufs=1) as w_pool,
                tc.tile_pool(name="xtp", bufs=2) as xt_pool,
                tc.tile_pool(name="rope", bufs=2) as rope_pool,
                tc.tile_pool(name="psA", bufs=2, space="PSUM") as psA,
            ):
                wq_t = w_pool.tile([128, HC, D_LOC], bf16, tag="wq")
                wk_t = w_pool.tile([128, HC, D_LOC], bf16, tag="wk")
                wv_t = w_pool.tile([128, HC, D_LOC], bf16, tag="wv")

                xt_view = xt_d[:, :].rearrange(
                    "(hc p) (sb f) -> sb p hc f", p=128, f=SB)
                wq_view = wqt_d[:, :].rearrange("(hc p) d -> p hc d", p=128)
                wk_view = wkt_d[:, :].rearrange("(hc p) d -> p hc d", p=128)
                wv_view = wvt_d[:, :].rearrange("(hc p) d -> p hc d", p=128)

                xt_blk0 = xt_pool.tile([128, HC, SB], bf16, tag="xt")
                # chunked loads, interleaved so hc=0.. arrives first and the
                # first projection matmuls start ~immediately (subtile deps);
                # streams ordered by first use: xt+wq, cos/sin, wk, wv
                for c0 in range(0, HC, 2):
                    csl = slice(c0, c0 + 2)
                    nc.sync.dma_start(xt_blk0[:, csl, :], xt_view[0][:, csl, :])
                    nc.sync.dma_start(wq_t[:, csl, :], wq_view[:, csl, :])
                nc.sync.dma_start(cos_t[:], cos_d[:, :])
                nc.sync.dma_start(sin_t[:], sin_d[:, :])
                for c0 in range(0, HC, 4):
                    csl = slice(c0, c0 + 4)
                    nc.sync.dma_start(wk_t[:, csl, :], wk_view[:, csl, :])
                for c0 in range(0, HC, 4):
                    csl = slice(c0, c0 + 4)
                    nc.sync.dma_start(wv_t[:, csl, :], wv_view[:, csl, :])
                nc.sync.dma_start(
                    mask_t[:], masks_d[:, :, :].rearrange("j p f -> p j f"))
                nc.sync.dma_start(ones_t[:], ones_d[:, :])

                for sb in range(n_sb):
                    ssl = slice(sb * SB, (sb + 1) * SB)
                    if sb == 0:
                        xt_blk = xt_blk0
                    else:
                        xt_blk = xt_pool.tile([128, HC, SB], bf16, tag="xt")
                        for c0 in range(0, HC, 4):
                            csl = slice(c0, c0 + 4)
                            nc.sync.dma_start(
                                xt_blk[:, csl, :], xt_view[sb][:, csl, :])

                    for w_tile, dst in ((wq_t, qT), (wk_t, kT)):
                        # hc-outer: PE consumes DMA chunks incrementally,
                        # 4 PSUM banks accumulate the 4 heads in parallel
                        ps_h = [psA.tile([128, SB], f32, tag="pqk", bufs=6,
                                         name=f"pqk{h}")
                                for h in range(NH_LOC)]
                        for hc in range(HC):
                            for h in range(NH_LOC):
                                hs = slice(h * 128, (h + 1) * 128)
                                nc.tensor.matmul(
                                    ps_h[h][:], w_tile[:, hc, hs],
                                    xt_blk[:, hc, :],
                                    start=(hc == 0), stop=(hc == HC - 1))
                        for h in range(NH_LOC):
                            # RoPE: dst = ps*cos + shuffle(ps)*sin_pm
                            ps = ps_h[h]
                            t_sw = rope_pool.tile([128, SB], f32, tag="sw")
                            nc.vector.stream_shuffle(t_sw[:], ps[:], _SHUF_MASK)
                            t_pr = rope_pool.tile([128, SB], f32, tag="pr")
                            nc.vector.tensor_mul(t_pr[:], t_sw[:], sin_t[:, ssl])
                            t_cs = rope_pool.tile([128, SB], f32, tag="cs")
                            nc.vector.tensor_mul(t_cs[:], ps[:], cos_t[:, ssl])
                            nc.vector.tensor_add(dst[:, h, ssl], t_cs[:], t_pr[:])

                    for i in range(n_sb):
                        st = sb * 4 + i
                        isl = slice(i * 128, (i + 1) * 128)
                        ps = psA.tile([128, D_LOC], f32, tag="pv", bufs=2)
                        for hc in range(HC):
                            nc.tensor.matmul(
                                ps[:], xt_blk[:, hc, isl], wv_t[:, hc, :],
                                start=(hc == 0), stop=(hc == HC - 1))
                        nc.scalar.copy(v_t[:, st, :], ps[:])

            # ------------- Phase B: attention, Phase C: out proj -------------
            with (
                tc.tile_pool(name="wo", bufs=1) as wo_pool,
                tc.tile_pool(name="ot", bufs=1) as ot_pool,
            ):
                wo_t = wo_pool.tile([128, NH_LOC, H], bf16, tag="wo")
                nc.sync.dma_start(
                    wo_t[:], wot_d[:, :].rearrange("(dc p) o -> p dc o", p=128))
                ot_t = ot_pool.tile([128, NH_LOC, S], bf16, tag="ot")

                with (
                    tc.tile_pool(name="pt", bufs=2) as pt_pool,
                    tc.tile_pool(name="scr", bufs=2) as scr_pool,
                    tc.tile_pool(name="rcp", bufs=2) as rcp_pool,
                    tc.tile_pool(name="stage", bufs=4) as stage_pool,
                    tc.tile_pool(name="psB", bufs=1, space="PSUM") as psB,
                ):
                    # phase-C work units (st, hb), emitted interleaved with
                    # phase B so the in-order PE has filler during exp waits
                    c_units = []

                    def emit_c_unit(use_scalar=False):
                        st, hb = c_units.pop(0)
                        stsl = slice(st * 128, (st + 1) * 128)
                        ps_c = psB.tile([128, SB], f32, tag="pc", bufs=2,
                                        name=f"pc_{st}_{hb}")
                        for dc in range(NH_LOC):
                            nc.tensor.matmul(
                                ps_c[:],
                                ot_t[:, dc, stsl],
                                wo_t[:, dc, hb * SB:(hb + 1) * SB],
                                start=(dc == 0), stop=(dc == NH_LOC - 1))
                        o_sb = stage_pool.tile([128, SB], f32, tag="st")
                        if use_scalar:
                            nc.scalar.copy(o_sb[:], ps_c[:])
                        else:
                            nc.vector.tensor_copy(o_sb[:], ps_c[:])
                        nc.sync.dma_start(
                            out_d[stsl, hb * SB:(hb + 1) * SB], o_sb[:])

                    tri = mask_t[:, 0, 0:128]  # keep f >= p triangle
                    for qb in range(n_sb):
                        qsl = slice(qb * SB, (qb + 1) * SB)
                        nkt = 4 * (qb + 1)
                        for h in range(NH_LOC):
                            hs = slice(h * 128, (h + 1) * 128)
                            blk = pt_pool.tile([128, 16, SB], bf16, tag="pt")
                            ps_o = psB.tile([128, SB], f32, tag="o", bufs=2)
                            for c0 in range(0, nkt, 4):
                                for kt in range(c0, c0 + 4):
                                    j = kt - 4 * qb
                                    off = 128 * j if j > 0 else 0
                                    W = SB - off
                                    ksl = slice(kt * 128, (kt + 1) * 128)
                                    ps_s = psB.tile(
                                        [128, SB], f32, tag="s", bufs=3)
                                    nc.tensor.matmul(
                                        ps_s[:, 0:W], kT[:, h, ksl],
                                        qT[:, h, qb * SB + off:(qb + 1) * SB],
                                        start=True, stop=True)
                                    nc.scalar.activation(
                                        blk[:, kt, off:SB], ps_s[:, 0:W], Exp)
                                    if j >= 0:
                                        nc.vector.tensor_mul(
                                            blk[:, kt, off:off + 128],
                                            blk[:, kt, off:off + 128], tri)
                                for kt in range(c0, c0 + 4):
                                    j = kt - 4 * qb
                                    off = 128 * j if j > 0 else 0
                                    nc.tensor.matmul(
                                        ps_o[:, off:SB], v_t[:, kt, hs],
                                        blk[:, kt, off:SB],
                                        start=(kt == 0), stop=(kt == nkt - 1))
                                # phase-C filler for the PE during exp waits;
                                # pace so the backlog lasts through qb=3
                                n_fill = 1 if qb == 3 else 2
                                for _ in range(n_fill):
                                    if c_units:
                                        emit_c_unit()

                            # sums: elementwise kt-tree on DVE (bf16), then
                            # one all-ones matmul reduces partitions+broadcasts
                            scr = scr_pool.tile([128, 12, SB], bf16, tag="scr")
                            nd = nkt - 4  # non-diagonal count
                            # fold diag j=1..3 into diag j=0 (valid suffixes)
                            d0 = nkt - 4 + 0
                            for j in range(1, 4):
                                o = 128 * j
                                nc.vector.tensor_add(
                                    blk[:, d0, o:SB], blk[:, d0, o:SB],
                                    blk[:, d0 + j, o:SB])
                            if nd == 0:
                                sums_src = blk[:, d0, :]
                            else:
                                # pairwise-halve the nd non-diag tiles
                                nc.vector.tensor_add(
                                    scr[:, 0:nd // 2, :],
                                    blk[:, 0:nd:2, :], blk[:, 1:nd:2, :])
                                m = nd // 2
                                base = 0
                                while m > 1:
                                    nb = base + m
                                    nc.vector.tensor_add(
                                        scr[:, nb:nb + m // 2, :],
                                        scr[:, base:base + m - 1:2, :],
                                        scr[:, base + 1:base + m:2, :])
                                    if m % 2:
                                        # carry odd leftover
                                        nc.vector.tensor_add(
                                            scr[:, nb, :], scr[:, nb, :],
                                            scr[:, base + m - 1, :])
                                    base, m = nb, m // 2
                                nc.vector.tensor_add(
                                    scr[:, base, :], scr[:, base, :],
                                    blk[:, d0, :])
                                sums_src = scr[:, base, :]
                            ps_sum = psB.tile([128, SB], f32, tag="sum",
                                              bufs=1)
                            nc.tensor.matmul(ps_sum[:], ones_t[:],
                                             sums_src, start=True, stop=True)
                            rcp = rcp_pool.tile([128, SB], f32, tag="rcp")
                            nc.vector.reciprocal_approx_fast(rcp[:], ps_sum[:])
                            nc.vector.tensor_mul(
                                ot_t[:, h, qsl], ps_o[:], rcp[:])
                        # this qb's output rows are now fully available
                        for st in range(qb * 4, qb * 4 + 4):
                            for hb in range(4):
                                c_units.append((st, hb))
                    drain_i = 0
                    while c_units:
                        emit_c_unit(use_scalar=(drain_i % 2 == 0))
                        drain_i += 1

    nc.compile()
    return nc


def _host_prep(x, freq_cos, freq_sin, wq, wk, wv, wo):
    """Build the 8 per-core input maps."""
    cos_bc = np.ascontiguousarray(freq_cos.T[_J_OF_P, :]).astype(np.float32)
    sin_pm = np.ascontiguousarray(
        freq_sin.T[_J_OF_P, :] * _SIGN_OF_P[:, None]).astype(np.float32)

    f = np.arange(SB)[None, :]
    p = np.arange(128)[:, None]
    masks = np.stack(
        [(f - 128 * j - p >= 0) for j in range(4)]).astype(_BF16)
    ones = np.ones((128, 128), dtype=_BF16)

    xt_b = [np.ascontiguousarray(x[b].T).astype(_BF16) for b in range(2)]

    in_maps = []
    for c in range(N_CORES):
        b, g = c // 4, c % 4
        rows = slice(g * D_LOC, (g + 1) * D_LOC)
        wq_g = wq[rows, :].reshape(NH_LOC, HD, H)[:, _P_NEW2OLD, :]
        wk_g = wk[rows, :].reshape(NH_LOC, HD, H)[:, _P_NEW2OLD, :]
        in_maps.append({
            "xt": xt_b[b],
            "wqt": np.ascontiguousarray(
                wq_g.reshape(D_LOC, H).T * SCALE).astype(_BF16),
            "wkt": np.ascontiguousarray(
                wk_g.reshape(D_LOC, H).T).astype(_BF16),
            "wvt": np.ascontiguousarray(wv[rows, :].T).astype(_BF16),
            "wot": np.ascontiguousarray(wo[:, rows].T).astype(_BF16),
            "cos_bc": cos_bc,
            "sin_pm": sin_pm,
            "masks": masks,
            "ones": ones,
        })
    return in_maps


def _kernel_np_fallback(x, freq_cos, freq_sin, attention_mask, wq, wk, wv, wo):
    """Numpy fallback (only used if attention_mask is nonzero)."""
    B = x.shape[0]
    hd = H // NHEADS
    q = (x @ wq.T).reshape(B, S, NHEADS, hd)
    k = (x @ wk.T).reshape(B, S, NHEADS, hd)
    v = (x @ wv.T).reshape(B, S, NHEADS, hd)

    def rope(t):
        x1, x2 = t[..., ::2], t[..., 1::2]
        c = freq_cos[None, :, None, :]
        s = freq_sin[None, :, None, :]
        o = np.empty_like(t)
        o[..., ::2] = x1 * c - x2 * s
        o[..., 1::2] = x1 * s + x2 * c
        return o

    q, k = rope(q), rope(k)
    q = q.transpose(0, 2, 1, 3)
    k = k.transpose(0, 2, 1, 3)
    v = v.transpose(0, 2, 1, 3)
    att = np.einsum("bhqd,bhkd->bhqk", q, k) / np.sqrt(hd) / (LAYER_INDEX + 1)
    att = att + attention_mask
    causal = np.triu(np.full((S, S), -1e30, dtype=att.dtype), k=1)
    att = att + causal[None, None]
    att = att - att.max(axis=-1, keepdims=True)
    att = np.exp(att)
    att = att / att.sum(axis=-1, keepdims=True)
    out = np.einsum("bhqk,bhkd->bhqd", att, v)
    out = out.transpose(0, 2, 1, 3).reshape(B, S, H)
    return (out @ wo.T).astype(np.float32)


def kernel(x, freq_cos, freq_sin, attention_mask, wq, wk, wv, wo, **extra):
    x = np.asarray(x, dtype=np.float32)
    freq_cos = np.asarray(freq_cos, dtype=np.float32)
    freq_sin = np.asarray(freq_sin, dtype=np.float32)
    attention_mask = np.asarray(attention_mask, dtype=np.float32)
    wq = np.asarray(wq, dtype=np.float32)
    wk = np.asarray(wk, dtype=np.float32)
    wv = np.asarray(wv, dtype=np.float32)
    wo = np.asarray(wo, dtype=np.float32)

    if attention_mask.any():
        # the device kernel folds the (all-zero) pad mask away
        return _kernel_np_fallback(
            x, freq_cos, freq_sin, attention_mask, wq, wk, wv, wo)

    from concourse.bass_utils import run_bass_kernel_spmd

    if "nc" not in _NC_CACHE:
        _NC_CACHE["nc"] = _build_nc()
    nc = _NC_CACHE["nc"]

    in_maps = _host_prep(x, freq_cos, freq_sin, wq, wk, wv, wo)
    res = run_bass_kernel_spmd(nc, in_maps, list(range(N_CORES)))

    out = np.zeros((2, S, H), dtype=np.float32)
    for c in range(N_CORES):
        out[c // 4] += res.results[c]["out_partial"]
    return out


# revision 15
# speedup vs baseline: 1.0522x; 1.0064x over previous
"""Trainium2 Bass kernel for causal multi-head attention (8-core SPMD).

Problem: B=2, S=2048, H=2048, 16 heads (hd=128), RoPE, causal mask,
layer-index scaling (/4), additive pad mask (zeros by construction).

Sharding: core c handles batch b=c//4 and head-group g=c%4 (4 heads).
wq/wk/wv column-parallel, wo row-parallel; host sums the 4 partial
outputs per batch.

Per-core dataflow (all feature-on-partition, "transposed" layouts):
  qT/kT [d=128, S] = w.T-tile @ xT      (PSUM accum over 16 H-chunks)
  RoPE on qT/kT via head-dim permutation chosen so the rotation pair
    sits 16 partitions apart inside each 32-partition quadrant
    (stream_shuffle does the swap in one DVE op)
  scoresT [k,q] tiles = kT-tile.T @ qT-block ; exp on ACT -> PT (bf16)
  diagonal tiles masked multiplicatively post-exp
  row sums via all-ones stationary matmul (broadcast over partitions)
  OT [d, q] += v-tile.T @ PT ; normalized by reciprocal(sums)
  out_partial = OT.T @ woT  (PSUM accum over the 4 local head chunks)

Matmuls run in bf16 (fp32 PSUM accumulation); softmax math in fp32.
"""
import math
import os
import sys

import numpy as np

for _p in ("/opt/trn_rl_repo", "/root/.axon_site/_ro/trn_rl_repo"):
    if os.path.isdir(_p) and _p not in sys.path:
        sys.path.append(_p)

import ml_dtypes

S = 2048
H = 2048
NHEADS = 16
HD = 128
NH_LOC = 4          # heads per core
D_LOC = NH_LOC * HD  # 512
LAYER_INDEX = 3
SCALE = 1.0 / (math.sqrt(HD) * (LAYER_INDEX + 1))
N_CORES = 8
SB = 512            # S-block (matmul moving free dim)
HC = H // 128       # contraction chunks

# head-dim permutation: RoPE pair (x1_j, x2_j) -> rows (qd*32 + j%16,
# qd*32 + 16 + j%16) with qd = j//16, so the swap is within-quadrant.
_P_NEW2OLD = np.zeros(HD, dtype=np.int64)
_J_OF_P = np.zeros(HD, dtype=np.int64)
_SIGN_OF_P = np.zeros(HD, dtype=np.float32)
for _p in range(HD):
    _qd, _r = _p // 32, _p % 32
    _j = _qd * 16 + (_r % 16)
    _P_NEW2OLD[_p] = 2 * _j + (1 if _r >= 16 else 0)
    _J_OF_P[_p] = _j
    _SIGN_OF_P[_p] = 1.0 if _r >= 16 else -1.0
_SHUF_MASK = [(i + 16) % 32 for i in range(32)]

_BF16 = ml_dtypes.bfloat16
_NC_CACHE = {}


def _build_nc():
    import concourse.bacc as bacc
    import concourse.mybir as mybir
    import concourse.tile as tile

    f32 = mybir.dt.float32
    bf16 = mybir.dt.bfloat16
    Exp = mybir.ActivationFunctionType.Exp

    nc = bacc.Bacc("TRN2", target_bir_lowering=False, debug=False)

    xt_d = nc.dram_tensor("xt", [H, S], bf16, kind="ExternalInput")
    wqt_d = nc.dram_tensor("wqt", [H, D_LOC], bf16, kind="ExternalInput")
    wkt_d = nc.dram_tensor("wkt", [H, D_LOC], bf16, kind="ExternalInput")
    wvt_d = nc.dram_tensor("wvt", [H, D_LOC], bf16, kind="ExternalInput")
    wot_d = nc.dram_tensor("wot", [D_LOC, H], bf16, kind="ExternalInput")
    cos_d = nc.dram_tensor("cos_bc", [128, S], f32, kind="ExternalInput")
    sin_d = nc.dram_tensor("sin_pm", [128, S], f32, kind="ExternalInput")
    masks_d = nc.dram_tensor("masks", [4, 128, SB], bf16, kind="ExternalInput")
    ones_d = nc.dram_tensor("ones", [128, 128], bf16, kind="ExternalInput")
    out_d = nc.dram_tensor("out_partial", [S, H], f32, kind="ExternalOutput")

    n_sb = S // SB       # 4
    n_st = S // 128      # 16

    with tile.TileContext(nc) as tc:
        with (
            tc.tile_pool(name="const", bufs=1) as const_pool,
            tc.tile_pool(name="qkv", bufs=1) as qkv_pool,
        ):
            cos_t = const_pool.tile([128, S], f32, tag="cos")
            sin_t = const_pool.tile([128, S], f32, tag="sin")
            mask_t = const_pool.tile([128, 4, SB], bf16, tag="mask")
            ones_t = const_pool.tile([128, 128], bf16, tag="ones")

            qT = qkv_pool.tile([128, NH_LOC, S], bf16, tag="qT")
            kT = qkv_pool.tile([128, NH_LOC, S], bf16, tag="kT")
            v_t = qkv_pool.tile([128, n_st, D_LOC], bf16, tag="v")

            # ---------------- Phase A: projections + RoPE ----------------
            with (
                tc.tile_pool(name="w", bufs=1) as w_pool,
                tc.tile_pool(name="xtp", bufs=2) as xt_pool,
                tc.tile_pool(name="rope", bufs=2) as rope_pool,
                tc.tile_pool(name="psA", bufs=2, space="PSUM") as psA,
            ):
                wq_t = w_pool.tile([128, HC, D_LOC], bf16, tag="wq")
                wk_t = w_pool.tile([128, HC, D_LOC], bf16, tag="wk")
                wv_t = w_pool.tile([128, HC, D_LOC], bf16, tag="wv")

                xt_view = xt_d[:, :].rearrange(
                    "(hc p) (sb f) -> sb p hc f", p=128, f=SB)
                wq_view = wqt_d[:, :].rearrange("(hc p) d -> p hc d", p=128)
                wk_view = wkt_d[:, :].rearrange("(hc p) d -> p hc d", p=128)
                wv_view = wvt_d[:, :].rearrange("(hc p) d -> p hc d", p=128)

                xt_blk0 = xt_pool.tile([128, HC, SB], bf16, tag="xt")
                # chunked loads, interleaved so hc=0.. arrives first and the
                # first projection matmuls start ~immediately (subtile deps);
                # streams ordered by first use: xt+wq, cos/sin, wk, wv
                for c0 in range(0, HC, 2):
                    csl = slice(c0, c0 + 2)
                    nc.sync.dma_start(xt_blk0[:, csl, :], xt_view[0][:, csl, :])
                    nc.sync.dma_start(wq_t[:, csl, :], wq_view[:, csl, :])
                nc.sync.dma_start(cos_t[:], cos_d[:, :])
                nc.sync.dma_start(sin_t[:], sin_d[:, :])
                for c0 in range(0, HC, 4):
                    csl = slice(c0, c0 + 4)
                    nc.sync.dma_start(wk_t[:, csl, :], wk_view[:, csl, :])
                for c0 in range(0, HC, 4):
                    csl = slice(c0, c0 + 4)
                    nc.sync.dma_start(wv_t[:, csl, :], wv_view[:, csl, :])
                nc.sync.dma_start(
                    mask_t[:], masks_d[:, :, :].rearrange("j p f -> p j f"))
                nc.sync.dma_start(ones_t[:], ones_d[:, :])

                for sb in range(n_sb):
                    ssl = slice(sb * SB, (sb + 1) * SB)
                    if sb == 0:
                        xt_blk = xt_blk0
                    else:
                        xt_blk = xt_pool.tile([128, HC, SB], bf16, tag="xt")
                        for c0 in range(0, HC, 4):
                            csl = slice(c0, c0 + 4)
                            nc.sync.dma_start(
                                xt_blk[:, csl, :], xt_view[sb][:, csl, :])

                    for w_tile, dst in ((wq_t, qT), (wk_t, kT)):
                        # hc-outer: PE consumes DMA chunks incrementally,
                        # 4 PSUM banks accumulate the 4 heads in parallel
                        ps_h = [psA.tile([128, SB], f32, tag="pqk", bufs=6,
                                         name=f"pqk{h}")
                                for h in range(NH_LOC)]
                        for hc in range(HC):
                            for h in range(NH_LOC):
                                hs = slice(h * 128, (h + 1) * 128)
                                nc.tensor.matmul(
                                    ps_h[h][:], w_tile[:, hc, hs],
                                    xt_blk[:, hc, :],
                                    start=(hc == 0), stop=(hc == HC - 1))
                        for h in range(NH_LOC):
                            # RoPE: dst = ps*cos + shuffle(ps)*sin_pm
                            ps = ps_h[h]
                            t_sw = rope_pool.tile([128, SB], f32, tag="sw")
                            nc.vector.stream_shuffle(t_sw[:], ps[:], _SHUF_MASK)
                            t_pr = rope_pool.tile([128, SB], f32, tag="pr")
                            nc.vector.tensor_mul(t_pr[:], t_sw[:], sin_t[:, ssl])
                            t_cs = rope_pool.tile([128, SB], f32, tag="cs")
                            nc.vector.tensor_mul(t_cs[:], ps[:], cos_t[:, ssl])
                            nc.vector.tensor_add(dst[:, h, ssl], t_cs[:], t_pr[:])

                    for i in range(n_sb):
                        st = sb * 4 + i
                        isl = slice(i * 128, (i + 1) * 128)
                        ps = psA.tile([128, D_LOC], f32, tag="pv", bufs=2)
                        for hc in range(HC):
                            nc.tensor.matmul(
                                ps[:], xt_blk[:, hc, isl], wv_t[:, hc, :],
                                start=(hc == 0), stop=(hc == HC - 1))
                        nc.scalar.copy(v_t[:, st, :], ps[:])

            # ------------- Phase B: attention, Phase C: out proj -------------
            with (
                tc.tile_pool(name="wo", bufs=1) as wo_pool,
                tc.tile_pool(name="ot", bufs=1) as ot_pool,
            ):
                wo_t = wo_pool.tile([128, NH_LOC, H], bf16, tag="wo")
                nc.sync.dma_start(
                    wo_t[:], wot_d[:, :].rearrange("(dc p) o -> p dc o", p=128))
                ot_t = ot_pool.tile([128, NH_LOC, S], bf16, tag="ot")

                with (
                    tc.tile_pool(name="pt", bufs=2) as pt_pool,
                    tc.tile_pool(name="scr", bufs=2) as scr_pool,
                    tc.tile_pool(name="rcp", bufs=2) as rcp_pool,
                    tc.tile_pool(name="stage", bufs=4) as stage_pool,
                    tc.tile_pool(name="psB", bufs=1, space="PSUM") as psB,
                ):
                    # phase-C work units (st, hb), emitted interleaved with
                    # phase B so the in-order PE has filler during exp waits
                    c_units = []

                    def emit_c_unit(use_scalar=False):
                        st, hb = c_units.pop(0)
                        stsl = slice(st * 128, (st + 1) * 128)
                        ps_c = psB.tile([128, SB], f32, tag="pc", bufs=2,
                                        name=f"pc_{st}_{hb}")
                        for dc in range(NH_LOC):
                            nc.tensor.matmul(
                                ps_c[:],
                                ot_t[:, dc, stsl],
                                wo_t[:, dc, hb * SB:(hb + 1) * SB],
                                start=(dc == 0), stop=(dc == NH_LOC - 1))
                        o_sb = stage_pool.tile([128, SB], f32, tag="st")
                        if use_scalar:
                            nc.scalar.copy(o_sb[:], ps_c[:])
                        else:
                            nc.vector.tensor_copy(o_sb[:], ps_c[:])
                        nc.sync.dma_start(
                            out_d[stsl, hb * SB:(hb + 1) * SB], o_sb[:])

                    tri = mask_t[:, 0, 0:128]  # keep f >= p triangle
                    for qb in range(n_sb):
                        qsl = slice(qb * SB, (qb + 1) * SB)
                        nkt = 4 * (qb + 1)
                        for h in range(NH_LOC):
                            hs = slice(h * 128, (h + 1) * 128)
                            blk = pt_pool.tile([128, 16, SB], bf16, tag="pt")
                            ps_o = psB.tile([128, SB], f32, tag="o", bufs=2)
                            for c0 in range(0, nkt, 4):
                                for kt in range(c0, c0 + 4):
                                    j = kt - 4 * qb
                                    off = 128 * j if j > 0 else 0
                                    W = SB - off
                                    ksl = slice(kt * 128, (kt + 1) * 128)
                                    ps_s = psB.tile(
                                        [128, SB], f32, tag="s", bufs=3)
                                    nc.tensor.matmul(
                                        ps_s[:, 0:W], kT[:, h, ksl],
                                        qT[:, h, qb * SB + off:(qb + 1) * SB],
                                        start=True, stop=True)
                                    nc.scalar.activation(
                                        blk[:, kt, off:SB], ps_s[:, 0:W], Exp)
                                    if j >= 0:
                                        nc.vector.tensor_mul(
                                            blk[:, kt, off:off + 128],
                                            blk[:, kt, off:off + 128], tri)
                                for kt in range(c0, c0 + 4):
                                    j = kt - 4 * qb
                                    off = 128 * j if j > 0 else 0
                                    nc.tensor.matmul(
                                        ps_o[:, off:SB], v_t[:, kt, hs],
                                        blk[:, kt, off:SB],
                                        start=(kt == 0), stop=(kt == nkt - 1))
                                # phase-C filler for the PE during exp waits;
                                # reserve most units for the ACT-bound qb=3
                                n_fill = 1 if qb < 3 else 2
                                for _ in range(n_fill):
                                    if c_units:
                                        emit_c_unit()

                            # sums: elementwise kt-tree on DVE (bf16), then
                            # one all-ones matmul reduces partitions+broadcasts
                            scr = scr_pool.tile([128, 12, SB], bf16, tag="scr")
                            nd = nkt - 4  # non-diagonal count
                            # fold diag j=1..3 into diag j=0 (valid suffixes)
                            d0 = nkt - 4 + 0
                            for j in range(1, 4):
                                o = 128 * j
                                nc.vector.tensor_add(
                                    blk[:, d0, o:SB], blk[:, d0, o:SB],
                                    blk[:, d0 + j, o:SB])
                            if nd == 0:
                                sums_src = blk[:, d0, :]
                            else:
                                # pairwise-halve the nd non-diag tiles
                                nc.vector.tensor_add(
                                    scr[:, 0:nd // 2, :],
                                    blk[:, 0:nd:2, :], blk[:, 1:nd:2, :])
                                m = nd // 2
                                base = 0
                                while m > 1:
                                    nb = base + m
                                    nc.vector.tensor_add(
                                        scr[:, nb:nb + m // 2, :],
                                        scr[:, base:base + m - 1:2, :],
                                        scr[:, base + 1:base + m:2, :])
                                    if m % 2:
                                        # carry odd leftover
                                        nc.vector.tensor_add(
                                            scr[:, nb, :], scr[:, nb, :],
                                            scr[:, base + m - 1, :])
                                    base, m = nb, m // 2
                                nc.vector.tensor_add(
                                    scr[:, base, :], scr[:, base, :],
                                    blk[:, d0, :])
                                sums_src = scr[:, base, :]
                            ps_sum = psB.tile([128, SB], f32, tag="sum",
                                              bufs=1)
                            nc.tensor.matmul(ps_sum[:], ones_t[:],
                                             sums_src, start=True, stop=True)
                            rcp = rcp_pool.tile([128, SB], f32, tag="rcp")
                            nc.vector.reciprocal_approx_fast(rcp[:], ps_sum[:])
                            nc.vector.tensor_mul(
                                ot_t[:, h, qsl], ps_o[:], rcp[:])
                        # this qb's output rows are now fully available
                        for st in range(qb * 4, qb * 4 + 4):
                            for hb in range(4):
                                c_units.append((st, hb))
                    drain_i = 0
                    while c_units:
                        emit_c_unit(use_scalar=(drain_i % 2 == 0))
                        drain_i += 1

    nc.compile()
    return nc


def _host_prep(x, freq_cos, freq_sin, wq, wk, wv, wo):
    """Build the 8 per-core input maps."""
    cos_bc = np.ascontiguousarray(freq_cos.T[_J_OF_P, :]).astype(np.float32)
    sin_pm = np.ascontiguousarray(
        freq_sin.T[_J_OF_P, :] * _SIGN_OF_P[:, None]).astype(np.float32)

    f = np.arange(SB)[None, :]
    p = np.arange(128)[:, None]
    masks = np.stack(
        [(f - 128 * j - p >= 0) for j in range(4)]).astype(_BF16)
    ones = np.ones((128, 128), dtype=_BF16)

    xt_b = [np.ascontiguousarray(x[b].T).astype(_BF16) for b in range(2)]

    in_maps = []
    for c in range(N_CORES):
        b, g = c // 4, c % 4
        rows = slice(g * D_LOC, (g + 1) * D_LOC)
        wq_g = wq[rows, :].reshape(NH_LOC, HD, H)[:, _P_NEW2OLD, :]
        wk_g = wk[rows, :].reshape(NH_LOC, HD, H)[:, _P_NEW2OLD, :]
        in_maps.append({
            "xt": xt_b[b],
            "wqt": np.ascontiguousarray(
                wq_g.reshape(D_LOC, H).T * SCALE).astype(_BF16),
            "wkt": np.ascontiguousarray(
                wk_g.reshape(D_LOC, H).T).astype(_BF16),
            "wvt": np.ascontiguousarray(wv[rows, :].T).astype(_BF16),
            "wot": np.ascontiguousarray(wo[:, rows].T).astype(_BF16),
            "cos_bc": cos_bc,
            "sin_pm": sin_pm,
            "masks": masks,
            "ones": ones,
        })
    return in_maps


def _kernel_np_fallback(x, freq_cos, freq_sin, attention_mask, wq, wk, wv, wo):
    """Numpy fallback (only used if attention_mask is nonzero)."""
    B = x.shape[0]
    hd = H // NHEADS
    q = (x @ wq.T).reshape(B, S, NHEADS, hd)
    k = (x @ wk.T).reshape(B, S, NHEADS, hd)
    v = (x @ wv.T).reshape(B, S, NHEADS, hd)

    def rope(t):
        x1, x2 = t[..., ::2], t[..., 1::2]
        c = freq_cos[None, :, None, :]
        s = freq_sin[None, :, None, :]
        o = np.empty_like(t)
        o[..., ::2] = x1 * c - x2 * s
        o[..., 1::2] = x1 * s + x2 * c
        return o

    q, k = rope(q), rope(k)
    q = q.transpose(0, 2, 1, 3)
    k = k.transpose(0, 2, 1, 3)
    v = v.transpose(0, 2, 1, 3)
    att = np.einsum("bhqd,bhkd->bhqk", q, k) / np.sqrt(hd) / (LAYER_INDEX + 1)
    att = att + attention_mask
    causal = np.triu(np.full((S, S), -1e30, dtype=att.dtype), k=1)
    att = att + causal[None, None]
    att = att - att.max(axis=-1, keepdims=True)
    att = np.exp(att)
    att = att / att.sum(axis=-1, keepdims=True)
    out = np.einsum("bhqk,bhkd->bhqd", att, v)
    out = out.transpose(0, 2, 1, 3).reshape(B, S, H)
    return (out @ wo.T).astype(np.float32)


def kernel(x, freq_cos, freq_sin, attention_mask, wq, wk, wv, wo, **extra):
    x = np.asarray(x, dtype=np.float32)
    freq_cos = np.asarray(freq_cos, dtype=np.float32)
    freq_sin = np.asarray(freq_sin, dtype=np.float32)
    attention_mask = np.asarray(attention_mask, dtype=np.float32)
    wq = np.asarray(wq, dtype=np.float32)
    wk = np.asarray(wk, dtype=np.float32)
    wv = np.asarray(wv, dtype=np.float32)
    wo = np.asarray(wo, dtype=np.float32)

    if attention_mask.any():
        # the device kernel folds the (all-zero) pad mask away
        return _kernel_np_fallback(
            x, freq_cos, freq_sin, attention_mask, wq, wk, wv, wo)

    from concourse.bass_utils import run_bass_kernel_spmd

    if "nc" not in _NC_CACHE:
        _NC_CACHE["nc"] = _build_nc()
    nc = _NC_CACHE["nc"]

    in_maps = _host_prep(x, freq_cos, freq_sin, wq, wk, wv, wo)
    res = run_bass_kernel_spmd(nc, in_maps, list(range(N_CORES)))

    out = np.zeros((2, S, H), dtype=np.float32)
    for c in range(N_CORES):
        out[c // 4] += res.results[c]["out_partial"]
    return out


# revision 16
# speedup vs baseline: 1.0564x; 1.0040x over previous
"""Trainium2 Bass kernel for causal multi-head attention (8-core SPMD).

Problem: B=2, S=2048, H=2048, 16 heads (hd=128), RoPE, causal mask,
layer-index scaling (/4), additive pad mask (zeros by construction).

Sharding: core c handles batch b=c//4 and head-group g=c%4 (4 heads).
wq/wk/wv column-parallel, wo row-parallel; host sums the 4 partial
outputs per batch.

Per-core dataflow (all feature-on-partition, "transposed" layouts):
  qT/kT [d=128, S] = w.T-tile @ xT      (PSUM accum over 16 H-chunks)
  RoPE on qT/kT via head-dim permutation chosen so the rotation pair
    sits 16 partitions apart inside each 32-partition quadrant
    (stream_shuffle does the swap in one DVE op)
  scoresT [k,q] tiles = kT-tile.T @ qT-block ; exp on ACT -> PT (bf16)
  diagonal tiles masked multiplicatively post-exp
  row sums via all-ones stationary matmul (broadcast over partitions)
  OT [d, q] += v-tile.T @ PT ; normalized by reciprocal(sums)
  out_partial = OT.T @ woT  (PSUM accum over the 4 local head chunks)

Matmuls run in bf16 (fp32 PSUM accumulation); softmax math in fp32.
"""
import math
import os
import sys

import numpy as np

for _p in ("/opt/trn_rl_repo", "/root/.axon_site/_ro/trn_rl_repo"):
    if os.path.isdir(_p) and _p not in sys.path:
        sys.path.append(_p)

import ml_dtypes

S = 2048
H = 2048
NHEADS = 16
HD = 128
NH_LOC = 4          # heads per core
D_LOC = NH_LOC * HD  # 512
LAYER_INDEX = 3
SCALE = 1.0 / (math.sqrt(HD) * (LAYER_INDEX + 1))
N_CORES = 8
SB = 512            # S-block (matmul moving free dim)
HC = H // 128       # contraction chunks

# head-dim permutation: RoPE pair (x1_j, x2_j) -> rows (qd*32 + j%16,
# qd*32 + 16 + j%16) with qd = j//16, so the swap is within-quadrant.
_P_NEW2OLD = np.zeros(HD, dtype=np.int64)
_J_OF_P = np.zeros(HD, dtype=np.int64)
_SIGN_OF_P = np.zeros(HD, dtype=np.float32)
for _p in range(HD):
    _qd, _r = _p // 32, _p % 32
    _j = _qd * 16 + (_r % 16)
    _P_NEW2OLD[_p] = 2 * _j + (1 if _r >= 16 else 0)
    _J_OF_P[_p] = _j
    _SIGN_OF_P[_p] = 1.0 if _r >= 16 else -1.0
_SHUF_MASK = [(i + 16) % 32 for i in range(32)]

_BF16 = ml_dtypes.bfloat16
_NC_CACHE = {}


def _build_nc():
    import concourse.bacc as bacc
    import concourse.mybir as mybir
    import concourse.tile as tile

    f32 = mybir.dt.float32
    bf16 = mybir.dt.bfloat16
    Exp = mybir.ActivationFunctionType.Exp

    nc = bacc.Bacc("TRN2", target_bir_lowering=False, debug=False)

    xt_d = nc.dram_tensor("xt", [H, S], bf16, kind="ExternalInput")
    wqt_d = nc.dram_tensor("wqt", [H, D_LOC], bf16, kind="ExternalInput")
    wkt_d = nc.dram_tensor("wkt", [H, D_LOC], bf16, kind="ExternalInput")
    wvt_d = nc.dram_tensor("wvt", [H, D_LOC], bf16, kind="ExternalInput")
    wot_d = nc.dram_tensor("wot", [D_LOC, H], bf16, kind="ExternalInput")
    cos_d = nc.dram_tensor("cos_bc", [128, S], bf16, kind="ExternalInput")
    sin_d = nc.dram_tensor("sin_pm", [128, S], bf16, kind="ExternalInput")
    tri_d = nc.dram_tensor("tri", [128, 128], bf16, kind="ExternalInput")
    ones_d = nc.dram_tensor("ones", [128, 128], bf16, kind="ExternalInput")
    out_d = nc.dram_tensor("out_partial", [S, H], f32, kind="ExternalOutput")

    n_sb = S // SB       # 4
    n_st = S // 128      # 16

    with tile.TileContext(nc) as tc:
        with (
            tc.tile_pool(name="const", bufs=1) as const_pool,
            tc.tile_pool(name="qkv", bufs=1) as qkv_pool,
        ):
            cos_t = const_pool.tile([128, S], bf16, tag="cos")
            sin_t = const_pool.tile([128, S], bf16, tag="sin")
            tri_t = const_pool.tile([128, 128], bf16, tag="tri")
            ones_t = const_pool.tile([128, 128], bf16, tag="ones")

            qT = qkv_pool.tile([128, NH_LOC, S], bf16, tag="qT")
            kT = qkv_pool.tile([128, NH_LOC, S], bf16, tag="kT")
            v_t = qkv_pool.tile([128, n_st, D_LOC], bf16, tag="v")

            # ---------------- Phase A: projections + RoPE ----------------
            with (
                tc.tile_pool(name="w", bufs=1) as w_pool,
                tc.tile_pool(name="xtp", bufs=2) as xt_pool,
                tc.tile_pool(name="rope", bufs=2) as rope_pool,
                tc.tile_pool(name="psA", bufs=2, space="PSUM") as psA,
            ):
                wq_t = w_pool.tile([128, HC, D_LOC], bf16, tag="wq")
                wk_t = w_pool.tile([128, HC, D_LOC], bf16, tag="wk")
                wv_t = w_pool.tile([128, HC, D_LOC], bf16, tag="wv")

                xt_view = xt_d[:, :].rearrange(
                    "(hc p) (sb f) -> sb p hc f", p=128, f=SB)
                wq_view = wqt_d[:, :].rearrange("(hc p) d -> p hc d", p=128)
                wk_view = wkt_d[:, :].rearrange("(hc p) d -> p hc d", p=128)
                wv_view = wvt_d[:, :].rearrange("(hc p) d -> p hc d", p=128)

                xt_blk0 = xt_pool.tile([128, HC, SB], bf16, tag="xt")
                # chunked loads, interleaved so hc=0.. arrives first and the
                # first projection matmuls start ~immediately (subtile deps);
                # streams ordered by first use: xt+wq, cos/sin, wk, wv
                chunks = [(0, 2), (2, 2), (4, 4), (8, 4), (12, 4)]
                for c0, w in chunks:
                    csl = slice(c0, c0 + w)
                    nc.sync.dma_start(xt_blk0[:, csl, :], xt_view[0][:, csl, :])
                    nc.sync.dma_start(wq_t[:, csl, :], wq_view[:, csl, :])
                nc.sync.dma_start(cos_t[:], cos_d[:, :])
                nc.sync.dma_start(sin_t[:], sin_d[:, :])
                for c0 in range(0, HC, 4):
                    csl = slice(c0, c0 + 4)
                    nc.sync.dma_start(wk_t[:, csl, :], wk_view[:, csl, :])
                for c0 in range(0, HC, 4):
                    csl = slice(c0, c0 + 4)
                    nc.sync.dma_start(wv_t[:, csl, :], wv_view[:, csl, :])
                nc.sync.dma_start(tri_t[:], tri_d[:, :])
                nc.sync.dma_start(ones_t[:], ones_d[:, :])

                for sb in range(n_sb):
                    ssl = slice(sb * SB, (sb + 1) * SB)
                    if sb == 0:
                        xt_blk = xt_blk0
                    else:
                        xt_blk = xt_pool.tile([128, HC, SB], bf16, tag="xt")
                        for c0 in range(0, HC, 4):
                            csl = slice(c0, c0 + 4)
                            nc.sync.dma_start(
                                xt_blk[:, csl, :], xt_view[sb][:, csl, :])

                    for w_tile, dst in ((wq_t, qT), (wk_t, kT)):
                        # hc-outer: PE consumes DMA chunks incrementally,
                        # 4 PSUM banks accumulate the 4 heads in parallel
                        ps_h = [psA.tile([128, SB], f32, tag="pqk", bufs=6,
                                         name=f"pqk{h}")
                                for h in range(NH_LOC)]
                        for hc in range(HC):
                            for h in range(NH_LOC):
                                hs = slice(h * 128, (h + 1) * 128)
                                nc.tensor.matmul(
                                    ps_h[h][:], w_tile[:, hc, hs],
                                    xt_blk[:, hc, :],
                                    start=(hc == 0), stop=(hc == HC - 1))
                        for h in range(NH_LOC):
                            # RoPE: dst = ps*cos + shuffle(ps)*sin_pm
                            ps = ps_h[h]
                            t_sw = rope_pool.tile([128, SB], f32, tag="sw")
                            nc.vector.stream_shuffle(t_sw[:], ps[:], _SHUF_MASK)
                            t_pr = rope_pool.tile([128, SB], f32, tag="pr")
                            nc.vector.tensor_mul(t_pr[:], t_sw[:], sin_t[:, ssl])
                            t_cs = rope_pool.tile([128, SB], f32, tag="cs")
                            nc.vector.tensor_mul(t_cs[:], ps[:], cos_t[:, ssl])
                            nc.vector.tensor_add(dst[:, h, ssl], t_cs[:], t_pr[:])

                    for i in range(n_sb):
                        st = sb * 4 + i
                        isl = slice(i * 128, (i + 1) * 128)
                        ps = psA.tile([128, D_LOC], f32, tag="pv", bufs=2)
                        for hc in range(HC):
                            nc.tensor.matmul(
                                ps[:], xt_blk[:, hc, isl], wv_t[:, hc, :],
                                start=(hc == 0), stop=(hc == HC - 1))
                        nc.scalar.copy(v_t[:, st, :], ps[:])

            # ------------- Phase B: attention, Phase C: out proj -------------
            with (
                tc.tile_pool(name="wo", bufs=1) as wo_pool,
                tc.tile_pool(name="ot", bufs=1) as ot_pool,
            ):
                wo_t = wo_pool.tile([128, NH_LOC, H], bf16, tag="wo")
                nc.sync.dma_start(
                    wo_t[:], wot_d[:, :].rearrange("(dc p) o -> p dc o", p=128))
                ot_t = ot_pool.tile([128, NH_LOC, S], bf16, tag="ot")

                with (
                    tc.tile_pool(name="pt", bufs=2) as pt_pool,
                    tc.tile_pool(name="scr", bufs=2) as scr_pool,
                    tc.tile_pool(name="rcp", bufs=2) as rcp_pool,
                    tc.tile_pool(name="stage", bufs=4) as stage_pool,
                    tc.tile_pool(name="psB", bufs=1, space="PSUM") as psB,
                ):
                    # phase-C work units (st, hb), emitted interleaved with
                    # phase B so the in-order PE has filler during exp waits
                    c_units = []

                    def emit_c_unit(use_scalar=False):
                        st, hb = c_units.pop(0)
                        stsl = slice(st * 128, (st + 1) * 128)
                        ps_c = psB.tile([128, SB], f32, tag="pc", bufs=2,
                                        name=f"pc_{st}_{hb}")
                        for dc in range(NH_LOC):
                            nc.tensor.matmul(
                                ps_c[:],
                                ot_t[:, dc, stsl],
                                wo_t[:, dc, hb * SB:(hb + 1) * SB],
                                start=(dc == 0), stop=(dc == NH_LOC - 1))
                        o_sb = stage_pool.tile([128, SB], f32, tag="st")
                        if use_scalar:
                            nc.scalar.copy(o_sb[:], ps_c[:])
                        else:
                            nc.vector.tensor_copy(o_sb[:], ps_c[:])
                        nc.sync.dma_start(
                            out_d[stsl, hb * SB:(hb + 1) * SB], o_sb[:])

                    tri = tri_t[:, :]  # keep f >= p triangle
                    for qb in range(n_sb):
                        qsl = slice(qb * SB, (qb + 1) * SB)
                        nkt = 4 * (qb + 1)
                        for h in range(NH_LOC):
                            hs = slice(h * 128, (h + 1) * 128)
                            blk = pt_pool.tile([128, 16, SB], bf16, tag="pt")
                            ps_o = psB.tile([128, SB], f32, tag="o", bufs=2)
                            for c0 in range(0, nkt, 4):
                                for kt in range(c0, c0 + 4):
                                    j = kt - 4 * qb
                                    off = 128 * j if j > 0 else 0
                                    W = SB - off
                                    ksl = slice(kt * 128, (kt + 1) * 128)
                                    ps_s = psB.tile(
                                        [128, SB], f32, tag="s", bufs=3)
                                    nc.tensor.matmul(
                                        ps_s[:, 0:W], kT[:, h, ksl],
                                        qT[:, h, qb * SB + off:(qb + 1) * SB],
                                        start=True, stop=True)
                                    nc.scalar.activation(
                                        blk[:, kt, off:SB], ps_s[:, 0:W], Exp)
                                    if j >= 0:
                                        nc.vector.tensor_mul(
                                            blk[:, kt, off:off + 128],
                                            blk[:, kt, off:off + 128], tri)
                                for kt in range(c0, c0 + 4):
                                    j = kt - 4 * qb
                                    off = 128 * j if j > 0 else 0
                                    nc.tensor.matmul(
                                        ps_o[:, off:SB], v_t[:, kt, hs],
                                        blk[:, kt, off:SB],
                                        start=(kt == 0), stop=(kt == nkt - 1))
                                # phase-C filler for the PE during exp waits;
                                # reserve most units for the ACT-bound qb=3
                                n_fill = 1 if qb < 3 else 2
                                for _ in range(n_fill):
                                    if c_units:
                                        emit_c_unit()

                            # sums: elementwise kt-tree on DVE (bf16), then
                            # one all-ones matmul reduces partitions+broadcasts
                            scr = scr_pool.tile([128, 12, SB], bf16, tag="scr")
                            nd = nkt - 4  # non-diagonal count
                            # fold diag j=1..3 into diag j=0 (valid suffixes)
                            d0 = nkt - 4 + 0
                            for j in range(1, 4):
                                o = 128 * j
                                nc.vector.tensor_add(
                                    blk[:, d0, o:SB], blk[:, d0, o:SB],
                                    blk[:, d0 + j, o:SB])
                            if nd == 0:
                                sums_src = blk[:, d0, :]
                            else:
                                # pairwise-halve the nd non-diag tiles
                                nc.vector.tensor_add(
                                    scr[:, 0:nd // 2, :],
                                    blk[:, 0:nd:2, :], blk[:, 1:nd:2, :])
                                m = nd // 2
                                base = 0
                                while m > 1:
                                    nb = base + m
                                    nc.vector.tensor_add(
                                        scr[:, nb:nb + m // 2, :],
                                        scr[:, base:base + m - 1:2, :],
                                        scr[:, base + 1:base + m:2, :])
                                    if m % 2:
                                        # carry odd leftover
                                        nc.vector.tensor_add(
                                            scr[:, nb, :], scr[:, nb, :],
                                            scr[:, base + m - 1, :])
                                    base, m = nb, m // 2
                                nc.vector.tensor_add(
                                    scr[:, base, :], scr[:, base, :],
                                    blk[:, d0, :])
                                sums_src = scr[:, base, :]
                            ps_sum = psB.tile([128, SB], f32, tag="sum",
                                              bufs=1)
                            nc.tensor.matmul(ps_sum[:], ones_t[:],
                                             sums_src, start=True, stop=True)
                            rcp = rcp_pool.tile([128, SB], f32, tag="rcp")
                            nc.vector.reciprocal_approx_fast(rcp[:], ps_sum[:])
                            nc.vector.tensor_mul(
                                ot_t[:, h, qsl], ps_o[:], rcp[:])
                        # this qb's output rows are now fully available
                        for st in range(qb * 4, qb * 4 + 4):
                            for hb in range(4):
                                c_units.append((st, hb))
                    drain_i = 0
                    while c_units:
                        emit_c_unit(use_scalar=(drain_i % 2 == 0))
                        drain_i += 1

    nc.compile()
    return nc


def _host_prep(x, freq_cos, freq_sin, wq, wk, wv, wo):
    """Build the 8 per-core input maps."""
    cos_bc = np.ascontiguousarray(freq_cos.T[_J_OF_P, :]).astype(np.float32)
    sin_pm = np.ascontiguousarray(
        freq_sin.T[_J_OF_P, :] * _SIGN_OF_P[:, None]).astype(np.float32)

    f = np.arange(128)[None, :]
    p = np.arange(128)[:, None]
    tri = (f - p >= 0).astype(_BF16)
    ones = np.ones((128, 128), dtype=_BF16)

    xt_b = [np.ascontiguousarray(x[b].T).astype(_BF16) for b in range(2)]

    in_maps = []
    for c in range(N_CORES):
        b, g = c // 4, c % 4
        rows = slice(g * D_LOC, (g + 1) * D_LOC)
        wq_g = wq[rows, :].reshape(NH_LOC, HD, H)[:, _P_NEW2OLD, :]
        wk_g = wk[rows, :].reshape(NH_LOC, HD, H)[:, _P_NEW2OLD, :]
        in_maps.append({
            "xt": xt_b[b],
            "wqt": np.ascontiguousarray(
                wq_g.reshape(D_LOC, H).T * SCALE).astype(_BF16),
            "wkt": np.ascontiguousarray(
                wk_g.reshape(D_LOC, H).T).astype(_BF16),
            "wvt": np.ascontiguousarray(wv[rows, :].T).astype(_BF16),
            "wot": np.ascontiguousarray(wo[:, rows].T).astype(_BF16),
            "cos_bc": cos_bc.astype(_BF16),
            "sin_pm": sin_pm.astype(_BF16),
            "tri": tri,
            "ones": ones,
        })
    return in_maps


def _kernel_np_fallback(x, freq_cos, freq_sin, attention_mask, wq, wk, wv, wo):
    """Numpy fallback (only used if attention_mask is nonzero)."""
    B = x.shape[0]
    hd = H // NHEADS
    q = (x @ wq.T).reshape(B, S, NHEADS, hd)
    k = (x @ wk.T).reshape(B, S, NHEADS, hd)
    v = (x @ wv.T).reshape(B, S, NHEADS, hd)

    def rope(t):
        x1, x2 = t[..., ::2], t[..., 1::2]
        c = freq_cos[None, :, None, :]
        s = freq_sin[None, :, None, :]
        o = np.empty_like(t)
        o[..., ::2] = x1 * c - x2 * s
        o[..., 1::2] = x1 * s + x2 * c
        return o

    q, k = rope(q), rope(k)
    q = q.transpose(0, 2, 1, 3)
    k = k.transpose(0, 2, 1, 3)
    v = v.transpose(0, 2, 1, 3)
    att = np.einsum("bhqd,bhkd->bhqk", q, k) / np.sqrt(hd) / (LAYER_INDEX + 1)
    att = att + attention_mask
    causal = np.triu(np.full((S, S), -1e30, dtype=att.dtype), k=1)
    att = att + causal[None, None]
    att = att - att.max(axis=-1, keepdims=True)
    att = np.exp(att)
    att = att / att.sum(axis=-1, keepdims=True)
    out = np.einsum("bhqk,bhkd->bhqd", att, v)
    out = out.transpose(0, 2, 1, 3).reshape(B, S, H)
    return (out @ wo.T).astype(np.float32)


def kernel(x, freq_cos, freq_sin, attention_mask, wq, wk, wv, wo, **extra):
    x = np.asarray(x, dtype=np.float32)
    freq_cos = np.asarray(freq_cos, dtype=np.float32)
    freq_sin = np.asarray(freq_sin, dtype=np.float32)
    attention_mask = np.asarray(attention_mask, dtype=np.float32)
    wq = np.asarray(wq, dtype=np.float32)
    wk = np.asarray(wk, dtype=np.float32)
    wv = np.asarray(wv, dtype=np.float32)
    wo = np.asarray(wo, dtype=np.float32)

    if attention_mask.any():
        # the device kernel folds the (all-zero) pad mask away
        return _kernel_np_fallback(
            x, freq_cos, freq_sin, attention_mask, wq, wk, wv, wo)

    from concourse.bass_utils import run_bass_kernel_spmd

    if "nc" not in _NC_CACHE:
        _NC_CACHE["nc"] = _build_nc()
    nc = _NC_CACHE["nc"]

    in_maps = _host_prep(x, freq_cos, freq_sin, wq, wk, wv, wo)
    res = run_bass_kernel_spmd(nc, in_maps, list(range(N_CORES)))

    out = np.zeros((2, S, H), dtype=np.float32)
    for c in range(N_CORES):
        out[c // 4] += res.results[c]["out_partial"]
    return out


# revision 17
# speedup vs baseline: 1.0647x; 1.0078x over previous
"""Trainium2 Bass kernel for causal multi-head attention (8-core SPMD).

Problem: B=2, S=2048, H=2048, 16 heads (hd=128), RoPE, causal mask,
layer-index scaling (/4), additive pad mask (zeros by construction).

Sharding: core c handles batch b=c//4 and head-group g=c%4 (4 heads).
wq/wk/wv column-parallel, wo row-parallel; host sums the 4 partial
outputs per batch.

Per-core dataflow (all feature-on-partition, "transposed" layouts):
  qT/kT [d=128, S] = w.T-tile @ xT      (PSUM accum over 16 H-chunks)
  RoPE on qT/kT via head-dim permutation chosen so the rotation pair
    sits 16 partitions apart inside each 32-partition quadrant
    (stream_shuffle does the swap in one DVE op)
  scoresT [k,q] tiles = kT-tile.T @ qT-block ; exp on ACT -> PT (bf16)
  diagonal tiles masked multiplicatively post-exp
  row sums via all-ones stationary matmul (broadcast over partitions)
  OT [d, q] += v-tile.T @ PT ; normalized by reciprocal(sums)
  out_partial = OT.T @ woT  (PSUM accum over the 4 local head chunks)

Matmuls run in bf16 (fp32 PSUM accumulation); softmax math in fp32.
"""
import math
import os
import sys

import numpy as np

for _p in ("/opt/trn_rl_repo", "/root/.axon_site/_ro/trn_rl_repo"):
    if os.path.isdir(_p) and _p not in sys.path:
        sys.path.append(_p)

import ml_dtypes

S = 2048
H = 2048
NHEADS = 16
HD = 128
NH_LOC = 4          # heads per core
D_LOC = NH_LOC * HD  # 512
LAYER_INDEX = 3
SCALE = 1.0 / (math.sqrt(HD) * (LAYER_INDEX + 1))
N_CORES = 8
SB = 512            # S-block (matmul moving free dim)
HC = H // 128       # contraction chunks

# head-dim permutation: RoPE pair (x1_j, x2_j) -> rows (qd*32 + j%16,
# qd*32 + 16 + j%16) with qd = j//16, so the swap is within-quadrant.
_P_NEW2OLD = np.zeros(HD, dtype=np.int64)
_J_OF_P = np.zeros(HD, dtype=np.int64)
_SIGN_OF_P = np.zeros(HD, dtype=np.float32)
for _p in range(HD):
    _qd, _r = _p // 32, _p % 32
    _j = _qd * 16 + (_r % 16)
    _P_NEW2OLD[_p] = 2 * _j + (1 if _r >= 16 else 0)
    _J_OF_P[_p] = _j
    _SIGN_OF_P[_p] = 1.0 if _r >= 16 else -1.0
_SHUF_MASK = [(i + 16) % 32 for i in range(32)]

_BF16 = ml_dtypes.bfloat16
_NC_CACHE = {}


def _build_nc():
    import concourse.bacc as bacc
    import concourse.mybir as mybir
    import concourse.tile as tile

    f32 = mybir.dt.float32
    bf16 = mybir.dt.bfloat16
    Exp = mybir.ActivationFunctionType.Exp

    nc = bacc.Bacc("TRN2", target_bir_lowering=False, debug=False)

    xt_d = nc.dram_tensor("xt", [H, S], bf16, kind="ExternalInput")
    wqt_d = nc.dram_tensor("wqt", [H, D_LOC], bf16, kind="ExternalInput")
    wkt_d = nc.dram_tensor("wkt", [H, D_LOC], bf16, kind="ExternalInput")
    wvt_d = nc.dram_tensor("wvt", [H, D_LOC], bf16, kind="ExternalInput")
    wot_d = nc.dram_tensor("wot", [D_LOC, H], bf16, kind="ExternalInput")
    cos_d = nc.dram_tensor("cos_bc", [128, S], bf16, kind="ExternalInput")
    sin_d = nc.dram_tensor("sin_pm", [128, S], bf16, kind="ExternalInput")
    tri_d = nc.dram_tensor("tri", [128, 128], bf16, kind="ExternalInput")
    ones_d = nc.dram_tensor("ones", [128, 128], bf16, kind="ExternalInput")
    out_d = nc.dram_tensor("out_partial", [S, H], f32, kind="ExternalOutput")

    n_sb = S // SB       # 4
    n_st = S // 128      # 16

    with tile.TileContext(nc) as tc:
        with (
            tc.tile_pool(name="const", bufs=1) as const_pool,
            tc.tile_pool(name="qkv", bufs=1) as qkv_pool,
        ):
            cos_t = const_pool.tile([128, S], bf16, tag="cos")
            sin_t = const_pool.tile([128, S], bf16, tag="sin")
            tri_t = const_pool.tile([128, 128], bf16, tag="tri")
            ones_t = const_pool.tile([128, 128], bf16, tag="ones")

            qT = qkv_pool.tile([128, NH_LOC, S], bf16, tag="qT")
            kT = qkv_pool.tile([128, NH_LOC, S], bf16, tag="kT")
            v_t = qkv_pool.tile([128, n_st, D_LOC], bf16, tag="v")

            # ---------------- Phase A: projections + RoPE ----------------
            with (
                tc.tile_pool(name="w", bufs=1) as w_pool,
                tc.tile_pool(name="xtp", bufs=2) as xt_pool,
                tc.tile_pool(name="rope", bufs=2) as rope_pool,
                tc.tile_pool(name="psA", bufs=2, space="PSUM") as psA,
            ):
                wq_t = w_pool.tile([128, HC, D_LOC], bf16, tag="wq")
                wk_t = w_pool.tile([128, HC, D_LOC], bf16, tag="wk")
                wv_t = w_pool.tile([128, HC, D_LOC], bf16, tag="wv")

                xt_view = xt_d[:, :].rearrange(
                    "(hc p) (sb f) -> sb p hc f", p=128, f=SB)
                wq_view = wqt_d[:, :].rearrange("(hc p) d -> p hc d", p=128)
                wk_view = wkt_d[:, :].rearrange("(hc p) d -> p hc d", p=128)
                wv_view = wvt_d[:, :].rearrange("(hc p) d -> p hc d", p=128)

                xt_blk0 = xt_pool.tile([128, HC, SB], bf16, tag="xt")
                # chunked loads, interleaved so hc=0.. arrives first and the
                # first projection matmuls start ~immediately (subtile deps);
                # streams ordered by first use: xt+wq, cos/sin, wk, wv
                chunks = [(0, 1), (1, 1), (2, 2), (4, 4), (8, 4), (12, 4)]
                for c0, w in chunks:
                    csl = slice(c0, c0 + w)
                    nc.sync.dma_start(xt_blk0[:, csl, :], xt_view[0][:, csl, :])
                    nc.sync.dma_start(wq_t[:, csl, :], wq_view[:, csl, :])
                nc.sync.dma_start(cos_t[:], cos_d[:, :])
                nc.sync.dma_start(sin_t[:], sin_d[:, :])
                for c0 in range(0, HC, 4):
                    csl = slice(c0, c0 + 4)
                    nc.sync.dma_start(wk_t[:, csl, :], wk_view[:, csl, :])
                for c0 in range(0, HC, 4):
                    csl = slice(c0, c0 + 4)
                    nc.sync.dma_start(wv_t[:, csl, :], wv_view[:, csl, :])
                nc.sync.dma_start(tri_t[:], tri_d[:, :])
                nc.sync.dma_start(ones_t[:], ones_d[:, :])

                for sb in range(n_sb):
                    ssl = slice(sb * SB, (sb + 1) * SB)
                    if sb == 0:
                        xt_blk = xt_blk0
                    else:
                        xt_blk = xt_pool.tile([128, HC, SB], bf16, tag="xt")
                        for c0 in range(0, HC, 4):
                            csl = slice(c0, c0 + 4)
                            nc.sync.dma_start(
                                xt_blk[:, csl, :], xt_view[sb][:, csl, :])

                    for w_tile, dst in ((wq_t, qT), (wk_t, kT)):
                        # hc-outer: PE consumes DMA chunks incrementally,
                        # 4 PSUM banks accumulate the 4 heads in parallel
                        ps_h = [psA.tile([128, SB], f32, tag="pqk", bufs=6,
                                         name=f"pqk{h}")
                                for h in range(NH_LOC)]
                        for hc in range(HC):
                            for h in range(NH_LOC):
                                hs = slice(h * 128, (h + 1) * 128)
                                nc.tensor.matmul(
                                    ps_h[h][:], w_tile[:, hc, hs],
                                    xt_blk[:, hc, :],
                                    start=(hc == 0), stop=(hc == HC - 1))
                        for h in range(NH_LOC):
                            # RoPE: dst = ps*cos + shuffle(ps)*sin_pm
                            ps = ps_h[h]
                            t_sw = rope_pool.tile([128, SB], f32, tag="sw")
                            nc.vector.stream_shuffle(t_sw[:], ps[:], _SHUF_MASK)
                            t_pr = rope_pool.tile([128, SB], f32, tag="pr")
                            nc.vector.tensor_mul(t_pr[:], t_sw[:], sin_t[:, ssl])
                            t_cs = rope_pool.tile([128, SB], f32, tag="cs")
                            nc.vector.tensor_mul(t_cs[:], ps[:], cos_t[:, ssl])
                            nc.vector.tensor_add(dst[:, h, ssl], t_cs[:], t_pr[:])

                    for i in range(n_sb):
                        st = sb * 4 + i
                        isl = slice(i * 128, (i + 1) * 128)
                        ps = psA.tile([128, D_LOC], f32, tag="pv", bufs=2)
                        for hc in range(HC):
                            nc.tensor.matmul(
                                ps[:], xt_blk[:, hc, isl], wv_t[:, hc, :],
                                start=(hc == 0), stop=(hc == HC - 1))
                        nc.scalar.copy(v_t[:, st, :], ps[:])

            # ------------- Phase B: attention, Phase C: out proj -------------
            with (
                tc.tile_pool(name="wo", bufs=1) as wo_pool,
                tc.tile_pool(name="ot", bufs=1) as ot_pool,
            ):
                wo_t = wo_pool.tile([128, NH_LOC, H], bf16, tag="wo")
                nc.sync.dma_start(
                    wo_t[:], wot_d[:, :].rearrange("(dc p) o -> p dc o", p=128))
                ot_t = ot_pool.tile([128, NH_LOC, S], bf16, tag="ot")

                with (
                    tc.tile_pool(name="pt", bufs=2) as pt_pool,
                    tc.tile_pool(name="scr", bufs=2) as scr_pool,
                    tc.tile_pool(name="rcp", bufs=2) as rcp_pool,
                    tc.tile_pool(name="stage", bufs=4) as stage_pool,
                    tc.tile_pool(name="psB", bufs=1, space="PSUM") as psB,
                ):
                    # phase-C work units (st, hb), emitted interleaved with
                    # phase B so the in-order PE has filler during exp waits
                    c_units = []

                    def emit_c_unit(use_scalar=False):
                        st, hb = c_units.pop(0)
                        stsl = slice(st * 128, (st + 1) * 128)
                        ps_c = psB.tile([128, SB], f32, tag="pc", bufs=2,
                                        name=f"pc_{st}_{hb}")
                        for dc in range(NH_LOC):
                            nc.tensor.matmul(
                                ps_c[:],
                                ot_t[:, dc, stsl],
                                wo_t[:, dc, hb * SB:(hb + 1) * SB],
                                start=(dc == 0), stop=(dc == NH_LOC - 1))
                        o_sb = stage_pool.tile([128, SB], f32, tag="st")
                        if use_scalar:
                            nc.scalar.copy(o_sb[:], ps_c[:])
                        else:
                            nc.vector.tensor_copy(o_sb[:], ps_c[:])
                        nc.sync.dma_start(
                            out_d[stsl, hb * SB:(hb + 1) * SB], o_sb[:])

                    tri = tri_t[:, :]  # keep f >= p triangle
                    for qb in range(n_sb):
                        qsl = slice(qb * SB, (qb + 1) * SB)
                        nkt = 4 * (qb + 1)
                        for h in range(NH_LOC):
                            hs = slice(h * 128, (h + 1) * 128)
                            blk = pt_pool.tile([128, 16, SB], bf16, tag="pt")
                            ps_o = psB.tile([128, SB], f32, tag="o", bufs=2)
                            for c0 in range(0, nkt, 4):
                                for kt in range(c0, c0 + 4):
                                    j = kt - 4 * qb
                                    off = 128 * j if j > 0 else 0
                                    W = SB - off
                                    ksl = slice(kt * 128, (kt + 1) * 128)
                                    ps_s = psB.tile(
                                        [128, SB], f32, tag="s", bufs=3)
                                    nc.tensor.matmul(
                                        ps_s[:, 0:W], kT[:, h, ksl],
                                        qT[:, h, qb * SB + off:(qb + 1) * SB],
                                        start=True, stop=True)
                                    nc.scalar.activation(
                                        blk[:, kt, off:SB], ps_s[:, 0:W], Exp)
                                    if j >= 0:
                                        nc.vector.tensor_mul(
                                            blk[:, kt, off:off + 128],
                                            blk[:, kt, off:off + 128], tri)
                                for kt in range(c0, c0 + 4):
                                    j = kt - 4 * qb
                                    off = 128 * j if j > 0 else 0
                                    nc.tensor.matmul(
                                        ps_o[:, off:SB], v_t[:, kt, hs],
                                        blk[:, kt, off:SB],
                                        start=(kt == 0), stop=(kt == nkt - 1))
                                # phase-C filler for the PE during exp waits;
                                # reserve most units for the ACT-bound qb=3
                                n_fill = 1 if qb < 3 else (1 if h < 2 else 3)
                                for _ in range(n_fill):
                                    if c_units:
                                        emit_c_unit()

                            if (qb, h) == (3, 3):
                                # last iteration: PE ones-matmul sums; a DVE
                                # tree here would sit exposed on the tail
                                ps_sum = psB.tile([128, SB], f32, tag="sum",
                                                  bufs=1)
                                for kt in range(nkt):
                                    nc.tensor.matmul(
                                        ps_sum[:], ones_t[:], blk[:, kt, :],
                                        start=(kt == 0), stop=(kt == nkt - 1))
                                rcp = rcp_pool.tile([128, SB], f32, tag="rcp")
                                nc.vector.reciprocal_approx_fast(
                                    rcp[:], ps_sum[:])
                                nc.vector.tensor_mul(
                                    ot_t[:, h, qsl], ps_o[:], rcp[:])
                                continue
                            # sums: elementwise kt-tree on DVE (bf16), then
                            # one all-ones matmul reduces partitions+broadcasts
                            scr = scr_pool.tile([128, 12, SB], bf16, tag="scr")
                            nd = nkt - 4  # non-diagonal count
                            # fold diag j=1..3 into diag j=0 (valid suffixes)
                            d0 = nkt - 4 + 0
                            for j in range(1, 4):
                                o = 128 * j
                                nc.vector.tensor_add(
                                    blk[:, d0, o:SB], blk[:, d0, o:SB],
                                    blk[:, d0 + j, o:SB])
                            if nd == 0:
                                sums_src = blk[:, d0, :]
                            else:
                                # pairwise-halve the nd non-diag tiles
                                nc.vector.tensor_add(
                                    scr[:, 0:nd // 2, :],
                                    blk[:, 0:nd:2, :], blk[:, 1:nd:2, :])
                                m = nd // 2
                                base = 0
                                while m > 1:
                                    nb = base + m
                                    nc.vector.tensor_add(
                                        scr[:, nb:nb + m // 2, :],
                                        scr[:, base:base + m - 1:2, :],
                                        scr[:, base + 1:base + m:2, :])
                                    if m % 2:
                                        # carry odd leftover
                                        nc.vector.tensor_add(
                                            scr[:, nb, :], scr[:, nb, :],
                                            scr[:, base + m - 1, :])
                                    base, m = nb, m // 2
                                nc.vector.tensor_add(
                                    scr[:, base, :], scr[:, base, :],
                                    blk[:, d0, :])
                                sums_src = scr[:, base, :]
                            ps_sum = psB.tile([128, SB], f32, tag="sum",
                                              bufs=1)
                            nc.tensor.matmul(ps_sum[:], ones_t[:],
                                             sums_src, start=True, stop=True)
                            rcp = rcp_pool.tile([128, SB], f32, tag="rcp")
                            nc.vector.reciprocal_approx_fast(rcp[:], ps_sum[:])
                            nc.vector.tensor_mul(
                                ot_t[:, h, qsl], ps_o[:], rcp[:])
                        # this qb's output rows are now fully available
                        for st in range(qb * 4, qb * 4 + 4):
                            for hb in range(4):
                                c_units.append((st, hb))
                    drain_i = 0
                    while c_units:
                        emit_c_unit(use_scalar=(drain_i % 2 == 0))
                        drain_i += 1

    nc.compile()
    return nc


def _host_prep(x, freq_cos, freq_sin, wq, wk, wv, wo):
    """Build the 8 per-core input maps."""
    cos_bc = np.ascontiguousarray(freq_cos.T[_J_OF_P, :]).astype(np.float32)
    sin_pm = np.ascontiguousarray(
        freq_sin.T[_J_OF_P, :] * _SIGN_OF_P[:, None]).astype(np.float32)

    f = np.arange(128)[None, :]
    p = np.arange(128)[:, None]
    tri = (f - p >= 0).astype(_BF16)
    ones = np.ones((128, 128), dtype=_BF16)

    xt_b = [np.ascontiguousarray(x[b].T).astype(_BF16) for b in range(2)]

    in_maps = []
    for c in range(N_CORES):
        b, g = c // 4, c % 4
        rows = slice(g * D_LOC, (g + 1) * D_LOC)
        wq_g = wq[rows, :].reshape(NH_LOC, HD, H)[:, _P_NEW2OLD, :]
        wk_g = wk[rows, :].reshape(NH_LOC, HD, H)[:, _P_NEW2OLD, :]
        in_maps.append({
            "xt": xt_b[b],
            "wqt": np.ascontiguousarray(
                wq_g.reshape(D_LOC, H).T * SCALE).astype(_BF16),
            "wkt": np.ascontiguousarray(
                wk_g.reshape(D_LOC, H).T).astype(_BF16),
            "wvt": np.ascontiguousarray(wv[rows, :].T).astype(_BF16),
            "wot": np.ascontiguousarray(wo[:, rows].T).astype(_BF16),
            "cos_bc": cos_bc.astype(_BF16),
            "sin_pm": sin_pm.astype(_BF16),
            "tri": tri,
            "ones": ones,
        })
    return in_maps


def _kernel_np_fallback(x, freq_cos, freq_sin, attention_mask, wq, wk, wv, wo):
    """Numpy fallback (only used if attention_mask is nonzero)."""
    B = x.shape[0]
    hd = H // NHEADS
    q = (x @ wq.T).reshape(B, S, NHEADS, hd)
    k = (x @ wk.T).reshape(B, S, NHEADS, hd)
    v = (x @ wv.T).reshape(B, S, NHEADS, hd)

    def rope(t):
        x1, x2 = t[..., ::2], t[..., 1::2]
        c = freq_cos[None, :, None, :]
        s = freq_sin[None, :, None, :]
        o = np.empty_like(t)
        o[..., ::2] = x1 * c - x2 * s
        o[..., 1::2] = x1 * s + x2 * c
        return o

    q, k = rope(q), rope(k)
    q = q.transpose(0, 2, 1, 3)
    k = k.transpose(0, 2, 1, 3)
    v = v.transpose(0, 2, 1, 3)
    att = np.einsum("bhqd,bhkd->bhqk", q, k) / np.sqrt(hd) / (LAYER_INDEX + 1)
    att = att + attention_mask
    causal = np.triu(np.full((S, S), -1e30, dtype=att.dtype), k=1)
    att = att + causal[None, None]
    att = att - att.max(axis=-1, keepdims=True)
    att = np.exp(att)
    att = att / att.sum(axis=-1, keepdims=True)
    out = np.einsum("bhqk,bhkd->bhqd", att, v)
    out = out.transpose(0, 2, 1, 3).reshape(B, S, H)
    return (out @ wo.T).astype(np.float32)


def kernel(x, freq_cos, freq_sin, attention_mask, wq, wk, wv, wo, **extra):
    x = np.asarray(x, dtype=np.float32)
    freq_cos = np.asarray(freq_cos, dtype=np.float32)
    freq_sin = np.asarray(freq_sin, dtype=np.float32)
    attention_mask = np.asarray(attention_mask, dtype=np.float32)
    wq = np.asarray(wq, dtype=np.float32)
    wk = np.asarray(wk, dtype=np.float32)
    wv = np.asarray(wv, dtype=np.float32)
    wo = np.asarray(wo, dtype=np.float32)

    if attention_mask.any():
        # the device kernel folds the (all-zero) pad mask away
        return _kernel_np_fallback(
            x, freq_cos, freq_sin, attention_mask, wq, wk, wv, wo)

    from concourse.bass_utils import run_bass_kernel_spmd

    if "nc" not in _NC_CACHE:
        _NC_CACHE["nc"] = _build_nc()
    nc = _NC_CACHE["nc"]

    in_maps = _host_prep(x, freq_cos, freq_sin, wq, wk, wv, wo)
    res = run_bass_kernel_spmd(nc, in_maps, list(range(N_CORES)))

    out = np.zeros((2, S, H), dtype=np.float32)
    for c in range(N_CORES):
        out[c // 4] += res.results[c]["out_partial"]
    return out
